# revision 68
# baseline (speedup 1.0000x reference)
"""Multi-head causal attention (B=4, T=2048, D=1024, H=16, DH=64) on 8 trn2 cores.

Sharding: core = 2*b + g  (b = batch 0..3, g = head-group 0..1, 8 heads each).
Each core computes q/k/v projections for its 8 heads, causal attention, and the
row-parallel slice of the output projection; the host sums the two partial
outputs per batch and adds the output bias.

Dataflow (matmuls bf16 -> fp32 PSUM), designed so the PE never waits on the
softmax epilogue and the DVE never runs a serial reciprocal:

  xT (D,T) host pre-transposed, loaded as 8 per-kt tiles in 128KB DMA chunks
  qT/kT  [2-head pairs, 128 x T]  = Wpair.T @ x.T      (PE, K=128 d-tiles)
  v      [T-tiles 128 x 520]      = x @ Wv (+ ones col per head for row sums)
  ST     [j-tile 128, i-chunk 512] = kT.T @ qT          (K=64, 2 heads packed
                                     in row groups 0/1 -> concurrent MM pair)
  causal diagonal tiles: -480 strict-upper-triangle added INSIDE the S
     accumulation group via a [128x128] matmul (utri.T @ (-480 I)), so the
     exp output is already masked -- no DVE mask multiplies.  (-480, not
     -1e9: the HW ACT exp spline NaNs far outside its table range.)
  expST  = exp(ST/8)  (ScalarE, scale fused)
  av     [65, 512] += v_aug.T @ expST  (row 64 = softmax denominator l)
  epilogue: per head av[0:64] -> base-0 bf16 staging tile; the 4 l-rows of a
     half-chunk gather at partitions 0/32/64/96 of one tile; one
     reciprocal_approx_fast inverts all of them; each row spreads to a
     partition-0 tile (the GPSIMD broadcast ucode ignores AP partition
     offsets on HW), partition_broadcast -> [64,512] bf16, one DVE multiply
     per head writes concatT.  The division cluster is deferred into the
     next unit so its DVE backlog never gates PE-feeding evacuations.
  y      [T x 1024] = concatT.T @ WoT_g slices (K=128 c-tiles, fp32 out),
     evacuated per 512-col half and DMA'd straight to DRAM.

The v/q/k projections and the output projection are cut into single-matmul
"filler" thunks and drained into the attention loop between J iterations with
deadline tracking, so the PE stays busy (HAM stays warm) while ScalarE
crunches exp.  PSUM: spair 2x2 banks + av 2 + proj/outproj shared pool 2 = 8.
"""

import math
from collections import deque

import numpy as np
import ml_dtypes

import concourse.bass as bass
import concourse.bacc as bacc
import concourse.mybir as mybir
import concourse.tile as tile
from concourse.vector_clock import ScopedClock
from concourse.bass_utils import run_bass_kernel_spmd

BF16 = mybir.dt.bfloat16
F32 = mybir.dt.float32
nbf16 = ml_dtypes.bfloat16

D = 1024
DH = 64
HL = 8          # heads per core
KD = D // 128   # d-tiles


# ---------------------------------------------------------------------------
# Walrus in this build rejects >1 sync-wait on SP TPB_CTRL instructions; split
# the TileContext tail-drain's sem waits into single-wait SP nops.
def _patched_drain_and_barrier(self, tick_clock, wait_clock):
    nc = self.nc
    collector = nc.sync.nop()
    wait_clock.add_sem_waits(
        collector.ins, ScopedClock({None: tick_clock.global_clock})
    )
    si = collector.ins.sync_info
    waits = list(si.on_wait) if si and si.on_wait else []
    if si is not None:
        si.on_wait = waits[:1]
    for w in waits[1:]:
        extra = nc.sync.nop()
        esi = extra.ins.sync_info
        if esi is None:
            extra.ins.sync_info = mybir.SyncInfo(on_wait=[w], on_update=[])
        else:
            esi.on_wait = [w]
    nc.sync.drain()
    nc.all_engine_barrier()
    popped = nc._tile_sem_poison_stack.pop()
    assert popped is self._sem_poison
    nc.clear_and_free_semaphores(list(self.sems.allocated().values()))
    nc.all_engine_barrier()


def _apply_tile_patch():
    tile.TileContext._drain_and_barrier = _patched_drain_and_barrier


class _Filler:
    """FIFO of emission thunks with position markers for deadline drains."""

    def __init__(self):
        self.q = deque()
        self.added = 0
        self.drained = 0

    def add(self, thunks):
        self.q.extend(thunks)
        self.added += len(thunks)
        return self.added  # marker: drain_until(marker) runs through here

    def drain(self, k):
        k = min(k, len(self.q))
        for _ in range(k):
            self.q.popleft()()
        self.drained += k

    def drain_until(self, marker):
        while self.drained < marker and self.q:
            self.q.popleft()()
            self.drained += 1

    def drain_all(self):
        self.drain(len(self.q))


# ---------------------------------------------------------------------------
def build_core_program(T=2048, mask_mode="causal", has_bias=False):
    """One-core program; same NEFF runs SPMD on all 8 cores."""
    import os as _os

    MASKMM = _os.environ.get("K_MASKMM", "1") == "1"
    PIPE = _os.environ.get("K_PIPE", "1") == "1"
    FP8QK = _os.environ.get("K_FP8QK", "0") == "1" and not has_bias
    _apply_tile_patch()
    NT = T // 128            # 128-row t-tiles
    CH = min(512, T)         # i-chunk width
    NCH = T // CH            # chunks
    JT = CH // 128           # j-tiles per chunk

    FP8 = mybir.dt.float8e4
    nc = bacc.Bacc("TRN2", target_bir_lowering=False, debug=False)
    xT_d = nc.declare_dram_parameter("xT", [D, T], BF16, isOutput=False)
    if FP8QK:
        x8_d = nc.declare_dram_parameter("x8", [D, T], FP8, isOutput=False)
        wq8_d = nc.declare_dram_parameter("wq8", [D, 512], FP8, isOutput=False)
        wk8_d = nc.declare_dram_parameter("wk8", [D, 512], FP8, isOutput=False)
    else:
        wq_d = nc.declare_dram_parameter("wq", [D, 512], BF16, isOutput=False)
        wk_d = nc.declare_dram_parameter("wk", [D, 512], BF16, isOutput=False)
    wv_d = nc.declare_dram_parameter("wv", [D, 512], BF16, isOutput=False)
    wo_d = nc.declare_dram_parameter("wo", [512, D], BF16, isOutput=False)
    utri_d = nc.declare_dram_parameter("utri", [128, 128], BF16, isOutput=False)
    negd_d = nc.declare_dram_parameter("negd", [128, 128], BF16, isOutput=False)
    tri_d = nc.declare_dram_parameter("tri", [128, 128], BF16, isOutput=False)
    if mask_mode == "general":
        mt_d = nc.declare_dram_parameter("maskT", [T, T], BF16, isOutput=False)
    if has_bias:
        wqb_d = nc.declare_dram_parameter("wqb", [1, 512], BF16, isOutput=False)
        wkb_d = nc.declare_dram_parameter("wkb", [1, 512], BF16, isOutput=False)
        wvb_d = nc.declare_dram_parameter("wvb", [1, 512], BF16, isOutput=False)
    y_d = nc.declare_dram_parameter("y", [T, D], F32, isOutput=True)

    Exp = mybir.ActivationFunctionType.Exp
    Ln = mybir.ActivationFunctionType.Ln

    with tile.TileContext(nc) as tc:
        with (
            tc.tile_pool(name="singles", bufs=1) as singles,
            tc.tile_pool(name="est", bufs=6) as est_pool,
            tc.tile_pool(name="zst", bufs=10) as zpool,
            tc.tile_pool(name="lp", bufs=8) as lpool,
            tc.tile_pool(name="lbcp", bufs=6) as lbcp,
            tc.tile_pool(name="ysbp", bufs=4) as ysbp,
            tc.tile_pool(name="ps_big", bufs=2, space="PSUM") as ps_big,
            tc.tile_pool(name="ps_av", bufs=2, space="PSUM") as ps_av,
            tc.tile_pool(name="ps_px", bufs=2, space="PSUM") as ps_px,
        ):
            # ---- loads -------------------------------------------------
            # per-queue DMA is ~34GB/s, so gate-critical tensors are split
            # into 128KB chunks across queues: the first v matmul needs only
            # wv[kt0] + xT[kt0], ready ~4us in.
            wv_sb = singles.tile([128, KD, 512], BF16, name="wv_sb")
            wv_r = wv_d[:, :].rearrange("(kt p) n -> p kt n", p=128)
            xT_r = xT_d[:, :].rearrange("(kt p) t -> p kt t", p=128)
            if not FP8QK:
                wq_sb = singles.tile([128, KD, 512], BF16, name="wq_sb")
                wq_r = wq_d[:, :].rearrange("(kt p) n -> p kt n", p=128)
                wk_sb = singles.tile([128, KD, 512], BF16, name="wk_sb")
                wk_r = wk_d[:, :].rearrange("(kt p) n -> p kt n", p=128)
            # xT as KD x 4 independent [128, 512] column-chunk tiles: the
            # prologue (v tiles 0-3 + q/k of chunk 0) touches only column
            # chunk 0, so the 16 gate-critical DMAs (wv per-kt + xT cc0
            # per-kt) land across all 16 queues in ~4us instead of the PE
            # stalling ~24us for the full 4MB.
            NCC = max(1, T // 512)
            xT_cc = [[None] * NCC for _ in range(KD)]
            # gate-critical first: wv + xT column-chunk 0 pairwise (16 DMAs
            # across the 16 queues), then q/k weights (prologue qk pieces),
            # then the remaining xT column chunks.
            for kt in range(KD):
                nc.sync.dma_start(
                    out=wv_sb[:, kt : kt + 1, :], in_=wv_r[:, kt : kt + 1, :]
                )
                xk = singles.tile([128, 512], BF16, name=f"xT{kt}_0")
                nc.sync.dma_start(out=xk, in_=xT_r[:, kt, 0:512])
                xT_cc[kt][0] = xk
            if not FP8QK:
                for kt2 in range(KD):
                    nc.sync.dma_start(
                        out=wq_sb[:, kt2 : kt2 + 1, :],
                        in_=wq_r[:, kt2 : kt2 + 1, :],
                    )
                    nc.sync.dma_start(
                        out=wk_sb[:, kt2 : kt2 + 1, :],
                        in_=wk_r[:, kt2 : kt2 + 1, :],
                    )
            for cc in range(1, NCC):
                for kt in range(KD):
                    xk = singles.tile([128, 512], BF16, name=f"xT{kt}_{cc}")
                    nc.sync.dma_start(
                        out=xk, in_=xT_r[:, kt, cc * 512 : (cc + 1) * 512]
                    )
                    xT_cc[kt][cc] = xk

            class _XT:
                """xT_sb[:, kt, sl] view shim over the column-chunk tiles."""

                def __getitem__(self, key):
                    p_sl, kt, t_sl = key
                    cc = t_sl.start // 512
                    assert t_sl.stop <= (cc + 1) * 512, (t_sl, cc)
                    return xT_cc[kt][cc][
                        p_sl, t_sl.start - cc * 512 : t_sl.stop - cc * 512
                    ]

            xT_sb = _XT()
            if FP8QK:
                x8_sb = singles.tile([128, KD, T], FP8, name="x8_sb")
                x8_r = x8_d[:, :].rearrange("(kt p) t -> p kt t", p=128)
                for q8 in range(8):
                    cw = T // 8
                    nc.sync.dma_start(
                        out=x8_sb[:, :, q8 * cw : (q8 + 1) * cw],
                        in_=x8_r[:, :, q8 * cw : (q8 + 1) * cw],
                    )
                wq8_sb = singles.tile([128, KD, 512], FP8, name="wq8_sb")
                nc.sync.dma_start(
                    out=wq8_sb,
                    in_=wq8_d[:, :].rearrange("(kt p) n -> p kt n", p=128),
                )
                wk8_sb = singles.tile([128, KD, 512], FP8, name="wk8_sb")
                nc.sync.dma_start(
                    out=wk8_sb,
                    in_=wk8_d[:, :].rearrange("(kt p) n -> p kt n", p=128),
                )
            wo_sb = singles.tile([128, 4, D], BF16, name="wo_sb")
            nc.sync.dma_start(
                out=wo_sb, in_=wo_d[:, :].rearrange("(ct p) o -> p ct o", p=128)
            )
            utri_sb = singles.tile([128, 128], BF16, name="utri_sb")
            nc.sync.dma_start(out=utri_sb, in_=utri_d[:, :])
            negd_sb = singles.tile([128, 128], BF16, name="negd_sb")
            nc.sync.dma_start(out=negd_sb, in_=negd_d[:, :])
            tri_sb = singles.tile([128, 128], BF16, name="tri_sb")
            nc.sync.dma_start(out=tri_sb, in_=tri_d[:, :])
            if has_bias:
                wqb_sb = singles.tile([1, 512], BF16, name="wqb_sb")
                nc.sync.dma_start(out=wqb_sb, in_=wqb_d[:, :])
                wkb_sb = singles.tile([1, 512], BF16, name="wkb_sb")
                nc.sync.dma_start(out=wkb_sb, in_=wkb_d[:, :])
                wvb_sb = singles.tile([1, 512], BF16, name="wvb_sb")
                nc.sync.dma_start(out=wvb_sb, in_=wvb_d[:, :])
                ones_sb = singles.tile([1, T], BF16, name="ones_sb")
                nc.vector.memset(ones_sb, 1.0)

            ones128 = singles.tile([1, 128], BF16, name="ones128")
            nc.vector.memset(ones128, 1.0)
            v_sb = singles.tile([128, NT, 8 * 65], BF16, name="v_sb")
            # ones columns for all tiles/heads in one strided memset
            v_all = v_sb[:, :, :].rearrange("p t (h x) -> p t h x", x=65)
            nc.vector.memset(v_all[:, :, :, 64:65], 1.0)

            qT_sb = singles.tile([128, 4, T], BF16, name="qT_sb")
            kT_sb = singles.tile([128, 4, T], BF16, name="kT_sb")
            concat_sb = singles.tile([128, 4, T], BF16, name="concat_sb")

            # ---- projection / outproj pieces (filler thunks) -----------
            def v_piece(tt):
                st = {}

                def mk(kt):
                    def f():
                        if kt == 0:
                            st[0] = ps_px.tile(
                                [128, 512], F32, name="proj_ps", tag="px"
                            )
                        nc.tensor.matmul(
                            st[0],
                            xT_sb[:, kt, tt * 128 : (tt + 1) * 128],
                            wv_sb[:, kt, :],
                            start=(kt == 0),
                            stop=(kt == KD - 1 and not has_bias),
                        )
                    return f

                thunks = [mk(kt) for kt in range(KD)]
                if has_bias:
                    def fb():
                        nc.tensor.matmul(
                            st[0],
                            ones_sb[0:1, tt * 128 : (tt + 1) * 128],
                            wvb_sb[0:1, :],
                            start=False,
                            stop=True,
                        )
                    thunks.append(fb)

                def evac():
                    v_view = v_sb[:, tt, :].rearrange("p (h x) -> p h x", x=65)
                    nc.vector.tensor_copy(
                        v_view[:, :, 0:64],
                        st[0][:, 0:512].rearrange("p (h x) -> p h x", x=64),
                    )
                thunks.append(evac)
                return thunks

            def qk_piece(kind, c, pr):
                dst = qT_sb if kind == "q" else kT_sb
                cs = slice(c * CH, (c + 1) * CH)
                st = {}

                if FP8QK:
                    w8 = wq8_sb if kind == "q" else wk8_sb

                    def mk8(j):
                        def f():
                            if j == 0:
                                st[0] = ps_px.tile(
                                    [128, 512], F32, name="proj_ps", tag="px"
                                )
                            # fp8 DoubleRow: contraction pairs (2j, 2j+1)
                            # k-tiles on the same partition -> K=256 per MM
                            nc.tensor.matmul(
                                st[0][:, 0:CH],
                                w8[:, 2 * j : 2 * j + 2, pr * 128 : (pr + 1) * 128],
                                x8_sb[:, 2 * j : 2 * j + 2, cs],
                                start=(j == 0),
                                stop=(j == KD // 2 - 1),
                                perf_mode=mybir.MatmulPerfMode.DoubleRow,
                            )
                        return f

                    thunks = [mk8(j) for j in range(KD // 2)]

                    def evac():
                        nc.vector.tensor_copy(dst[:, pr, cs], st[0][:, 0:CH])
                    thunks.append(evac)
                    return thunks

                w_sb = wq_sb if kind == "q" else wk_sb

                def mk(kt):
                    def f():
                        if kt == 0:
                            st[0] = ps_px.tile(
                                [128, 512], F32, name="proj_ps", tag="px"
                            )
                        nc.tensor.matmul(
                            st[0][:, 0:CH],
                            w_sb[:, kt, pr * 128 : (pr + 1) * 128],
                            xT_sb[:, kt, cs],
                            start=(kt == 0),
                            stop=(kt == KD - 1 and not has_bias),
                        )
                    return f

                thunks = [mk(kt) for kt in range(KD)]
                if has_bias:
                    b_sb = wqb_sb if kind == "q" else wkb_sb

                    def fb():
                        nc.tensor.matmul(
                            st[0][:, 0:CH],
                            b_sb[0:1, pr * 128 : (pr + 1) * 128],
                            ones_sb[0:1, cs],
                            start=False,
                            stop=True,
                        )
                    thunks.append(fb)

                def evac():
                    nc.vector.tensor_copy(dst[:, pr, cs], st[0][:, 0:CH])
                thunks.append(evac)
                return thunks

            def outproj_piece(it):
                # ct-outer order: the two oc matmuls of each ct share the
                # same stationary operand (concat tile) back-to-back
                st = {}
                thunks = []

                def mk(ct, oc):
                    def f():
                        if ct == 0:
                            st[oc] = ps_px.tile(
                                [128, 512], F32, name="y_ps", tag="px"
                            )
                        nc.tensor.matmul(
                            st[oc],
                            concat_sb[:, ct, it * 128 : (it + 1) * 128],
                            wo_sb[:, ct, oc * 512 : (oc + 1) * 512],
                            start=(ct == 0),
                            stop=(ct == 3),
                        )
                    return f

                for ct in range(4):
                    thunks += [mk(ct, 0), mk(ct, 1)]

                def ev(oc):
                    def f():
                        y_sb = ysbp.tile([128, 512], F32, name="y_sb", tag="ysb")
                        nc.vector.tensor_copy(y_sb, st[oc])
                        nc.sync.dma_start(
                            out=y_d[
                                it * 128 : (it + 1) * 128,
                                oc * 512 : (oc + 1) * 512,
                            ],
                            in_=y_sb,
                        )
                    return f

                thunks += [ev(0), ev(1)]
                return thunks

            # ---- attention unit ----------------------------------------
            if mask_mode == "general":
                _mt_cm = tc.tile_pool(name="mtiles", bufs=NT + 2)
                mt_pool = _mt_cm.__enter__()

            def emit_unit(c, pr, filler, Lc, pending=None):
                cs = slice(c * CH, (c + 1) * CH)
                n_j = (c + 1) * JT if mask_mode == "causal" else NT
                if mask_mode == "general":
                    m_tiles = []
                    for J in range(n_j):
                        mt = mt_pool.tile([128, 512], BF16, name="mt", tag="mt")
                        nc.sync.dma_start(
                            out=mt[:, :CH], in_=mt_d[J * 128 : (J + 1) * 128, cs]
                        )
                        m_tiles.append(mt)
                av_t = [
                    ps_av.tile([65, 512], F32, name="av", tag="av")
                    for _ in range(2)
                ]
                s_tiles = {}

                def emit_S(J):
                    r = J - c * JT
                    diag = MASKMM and mask_mode == "causal" and 0 <= r < JT
                    off = max(0, r) * 128 if mask_mode == "causal" else 0
                    w = CH - off
                    spair = ps_big.tile([128, 1024], F32, name="spair", tag="big")
                    # head A at [off, CH); head B packed at [512, 512+w) so
                    # the exp range [off, 512+w) is gap-free.  Emit the two
                    # K=64 matmuls back-to-back: row groups 0/1 -> the PE
                    # runs them concurrently.
                    for hh in range(2):
                        hs = slice(hh * 64, (hh + 1) * 64)
                        dst = (
                            spair[:, off:CH] if hh == 0 else spair[:, 512 : 512 + w]
                        )
                        nc.tensor.matmul(
                            dst,
                            kT_sb[hs, pr, J * 128 : (J + 1) * 128],
                            qT_sb[hs, pr, c * CH + off : (c + 1) * CH],
                            start=True,
                            stop=not diag,
                            skip_group_check=diag,
                        )
                    if diag:
                        # accumulate -480 on the strict upper triangle of the
                        # diagonal 128x128 square: out[m,n] += -480*utri[n,m]
                        for hh in range(2):
                            d0 = off if hh == 0 else 512
                            nc.tensor.matmul(
                                spair[:, d0 : d0 + 128],
                                utri_sb,
                                negd_sb,
                                start=False,
                                stop=True,
                                skip_group_check=True,
                            )
                    s_tiles[J] = (spair, off)

                if PIPE:
                    pace = max(1, math.ceil(len(filler.q) / max(1, n_j)))
                else:
                    filler.drain_all()
                    pace = 0
                emit_S(0)
                for J in range(n_j):
                    filler.drain(pace)
                    if J + 1 < n_j:
                        emit_S(J + 1)
                    spair, off = s_tiles.pop(J)
                    w = CH - off
                    b_sl = [slice(off, CH), slice(512, 512 + w)]
                    e_pair = est_pool.tile([128, 1024], BF16, name="e_t", tag="e")
                    # with fp8 q/k the weights carry a x64 scale each, so S
                    # arrives x4096; fold the exact 2^-12 descale into exp
                    nc.scalar.activation(
                        e_pair[:, off : 512 + w],
                        spair[:, off : 512 + w],
                        Exp,
                        scale=0.125 / 4096.0 if FP8QK else 0.125,
                    )
                    r = J - c * JT
                    if (
                        not MASKMM
                        and mask_mode == "causal"
                        and 0 <= r < JT
                    ):
                        for hh in range(2):
                            d0 = b_sl[hh].start
                            nc.vector.tensor_mul(
                                e_pair[:, d0 : d0 + 128],
                                e_pair[:, d0 : d0 + 128],
                                tri_sb,
                            )
                    if mask_mode == "general":
                        for hh in range(2):
                            nc.vector.tensor_mul(
                                e_pair[:, b_sl[hh]],
                                e_pair[:, b_sl[hh]],
                                m_tiles[J][:, :CH],
                            )
                    for hh in range(2):
                        h = 2 * pr + hh
                        nc.tensor.matmul(
                            av_t[hh][:, off:CH],
                            v_sb[:, J, h * 65 : (h + 1) * 65],
                            e_pair[:, b_sl[hh]],
                            start=(J == 0),
                            stop=(J == n_j - 1),
                        )
                # epilogue: evacuate each head to a base-0 staging tile and
                # the two l rows into the half-chunk gather tile at
                # quadrant-aligned partitions (engine writes must start at
                # partition 0/32/64/96).
                zs = []
                # high priority: these copies free the av PSUM pair that the
                # NEXT unit's first AV matmul WAR-waits on; jump them ahead
                # of the division cluster in the DVE queue
                with tc.high_priority():
                    for hh in range(2):
                        z = zpool.tile([64, 512], BF16, name="z", tag="z")
                        nc.vector.tensor_copy(z[:, :CH], av_t[hh][0:64, :CH])
                        zs.append(z)
                        p0 = 64 * (pr % 2) + 32 * hh
                        nc.vector.tensor_copy(
                            Lc[p0 : p0 + 1, :CH], av_t[hh][64:65, :CH]
                        )
                if pending is not None:
                    # previous half-chunk's softmax-division cluster: emitted
                    # after this unit's drains and PSUM-freeing copies so its
                    # DVE backlog gates as little PE work as possible
                    pending()
                if mask_mode == "general":
                    del m_tiles
                return zs

            # ---- schedule ----------------------------------------------
            filler = _Filler()
            units = [(c, pr) for c in range(NCH) for pr in range(4)]

            # prologue: v tiles for chunk 0 (all tiles unless causal), q/k for
            # the first two units
            n_v_pro = JT if mask_mode == "causal" else NT
            for tt in range(n_v_pro):
                for th in v_piece(tt):
                    th()
            for c, pr in units[: min(2, len(units))]:
                for th in qk_piece("q", c, pr):
                    th()
                for th in qk_piece("k", c, pr):
                    th()

            qk_markers = {}
            v_markers = {}
            qk_stream = units[2:]
            v_next = n_v_pro  # next v tile to enqueue

            def make_division(c, z_pair, Lc):
                """Closure emitting the half-chunk softmax-division cluster
                (on DVE; the ACT Ln/Exp route flip-flops activation table
                sets with the attention Exp -- 1.3us reload each time).  The
                4 l rows sit at partitions 0/32/64/96; in-between lanes hold
                1.0 and are never read."""
                prs = list(z_pair.keys())
                # the very last division runs at drain time with ScalarE
                # still crunching the final unit's exp backlog -- spread the
                # Linv rows on the (then-idle) DVE instead so the tail
                # doesn't wait on the ACT queue
                spread_dve = c == NCH - 1 and prs[-1] == 3

                def emit():
                    Linv = lpool.tile([128, 512], F32, name="Linv", tag="linv")
                    # custom-DVE bit-trick reciprocal, ~5x faster than the
                    # iterative divide; HW-verified on this exact [0:97] AP
                    nc.vector.reciprocal_approx_fast(
                        Linv[0:97, :CH], Lc[0:97, :CH]
                    )
                    cs = slice(c * CH, (c + 1) * CH)
                    for pr2 in prs:
                        for hh in range(2):
                            p0 = 64 * (pr2 % 2) + 32 * hh
                            # the GPSIMD broadcast ucode ignores AP partition
                            # offsets on HW (reads p0, writes from p0), so
                            # spread each Linv row to a partition-0 tile
                            # first; row 0 can be read in place.  The spreads
                            # run on ScalarE (its copy shares the exp table
                            # set) to keep the DVE queue short here -- these
                            # only gate the deferred broadcasts.
                            li = lpool.tile([1, 512], BF16, name="li", tag="li")
                            if spread_dve:
                                nc.vector.tensor_copy(
                                    li[0:1, :CH], Linv[p0 : p0 + 1, :CH]
                                )
                            else:
                                nc.scalar.copy(
                                    li[0:1, :CH], Linv[p0 : p0 + 1, :CH]
                                )
                            li_ap = li[0:1, :CH]
                            hs = slice(hh * 64, (hh + 1) * 64)
                            if spread_dve:
                                # drain time: the PE is idle and ps_px free;
                                # broadcast via a K=1 ones-matmul (213ns)
                                # instead of 4 serial 1.1us GPSIMD broadcasts
                                lbc_ps = ps_px.tile(
                                    [128, 512], F32, name="lbc_ps", tag="px"
                                )
                                nc.tensor.matmul(
                                    lbc_ps[0:64, :CH],
                                    ones128[0:1, 0:64],
                                    li_ap,
                                    start=True,
                                    stop=True,
                                )
                                nc.vector.tensor_mul(
                                    concat_sb[hs, pr2, cs],
                                    z_pair[pr2][hh][:, :CH],
                                    lbc_ps[0:64, :CH],
                                )
                            else:
                                lbc = lbcp.tile(
                                    [64, 512], BF16, name="lbc", tag="lbc"
                                )
                                nc.gpsimd.partition_broadcast(
                                    lbc[:, :CH], li_ap, channels=64
                                )
                                nc.vector.tensor_mul(
                                    concat_sb[hs, pr2, cs],
                                    z_pair[pr2][hh][:, :CH],
                                    lbc[:, :CH],
                                )

                return emit

            pending_div = None
            for u, (c, pr) in enumerate(units):
                # enqueue filler due soon (outproj last so its matmuls drain
                # late in the J loop, after the previous half-chunk's
                # division cluster has emitted its concat multiplies)
                if u < len(qk_stream):
                    c2, pr2 = qk_stream[u]
                    m = filler.add(qk_piece("q", c2, pr2))
                    m = filler.add(qk_piece("k", c2, pr2))
                    qk_markers[(c2, pr2)] = m
                if v_next < NT and u < NT - JT:
                    m = filler.add(v_piece(v_next))
                    v_markers[v_next] = m
                    v_next += 1
                # outproj tile u-5: shifted one unit past the (c-1, pr-1)
                # ready point so its matmuls are always emitted after the
                # (deferred) division cluster that writes its concat inputs
                if 0 <= u - 5 < (NCH - 1) * JT:
                    filler.add(outproj_piece(u - 5))
                if u == len(units) - 1 and (NCH - 1) * JT - 1 >= 0:
                    # last unit also drains the final previous-chunk tile
                    filler.add(outproj_piece((NCH - 1) * JT - 1))

                # deadlines: q/k of this unit, v tiles of this chunk
                if (c, pr) in qk_markers:
                    filler.drain_until(qk_markers[(c, pr)])
                vt_needed = (c + 1) * JT - 1 if mask_mode == "causal" else NT - 1
                if vt_needed in v_markers:
                    filler.drain_until(v_markers[vt_needed])

                if pr % 2 == 0:
                    Lc = lpool.tile([128, 512], F32, name="Lc", tag="lc")
                    # initialize so the reciprocal over [0:97] never sees
                    # garbage; same queue as the l-copies so ordering is by
                    # emission
                    nc.vector.memset(Lc, 1.0)
                    z_pair = {}
                z_pair[pr] = emit_unit(c, pr, filler, Lc, pending=pending_div)
                pending_div = None

                if pr % 2 == 1:
                    pending_div = make_division(c, z_pair, Lc)

            # drain: final division, remaining filler, last outproj tiles
            if pending_div is not None:
                pending_div()
            filler.drain_all()
            for it in range((NCH - 1) * JT, NCH * JT):
                for th in outproj_piece(it):
                    th()
            if mask_mode == "general":
                _mt_cm.__exit__(None, None, None)
    nc.finalize()
    return nc


# ---------------------------------------------------------------------------
# Optional NTFF profiling (test.py sets TRACE=True). Registers the missing
# antenv.axon_hooks module so run_bass_kernel_spmd's trace path works.
TRACE = False
LAST_EXEC_TIME_NS = None
LAST_RESULTS = None


def _ensure_ntff_hook():
    import sys as _sys
    import types as _types

    if "antenv.axon_hooks" in _sys.modules:
        return
    mod = _types.ModuleType("antenv.axon_hooks")
    state = {"hook": None}
    mod.set_axon_ntff_profile_hook = lambda h: state.__setitem__("hook", h)
    mod.get_axon_ntff_profile_hook = lambda: state["hook"]
    _sys.modules["antenv.axon_hooks"] = mod
    import antenv

    antenv.axon_hooks = mod
    try:
        from trn_agent_boot.trn_boot import _ntff_profile_via_ctypes

        hook = _ntff_profile_via_ctypes("/opt/axon/libaxon_pjrt.so")
        if hook is not None:
            mod.set_axon_ntff_profile_hook(hook)
    except Exception:
        pass


_PROGRAM_CACHE = {}


def _get_program(T, mask_mode, has_bias):
    key = (T, mask_mode, has_bias)
    if key not in _PROGRAM_CACHE:
        _PROGRAM_CACHE[key] = build_core_program(T, mask_mode, has_bias)
    return _PROGRAM_CACHE[key]


def _mask_mode_of(mask):
    m = np.asarray(mask)
    if m.all():
        return "full"
    T = m.shape[0]
    tril = np.tril(np.ones((T, T), dtype=bool))
    if np.array_equal(m.astype(bool), tril):
        return "causal"
    return "general"


def kernel(x, mask, Wq, bq, Wk, bk, Wv, bv, Wo, bo):
    x = np.asarray(x)
    B, T, D_ = x.shape
    H = Wq.shape[0]
    assert D_ == D and H == 16
    mask_mode = _mask_mode_of(mask)
    has_bias = bool(
        np.any(np.asarray(bq)) or np.any(np.asarray(bk)) or np.any(np.asarray(bv))
    )
    nc = _get_program(T, mask_mode, has_bias)

    import os as _os

    fp8qk = _os.environ.get("K_FP8QK", "0") == "1" and not has_bias
    utri = np.triu(np.ones((128, 128), dtype=np.float32), 1).astype(nbf16)
    # -480: large enough that exp((S-480)/8) ~ 4e-26 ~ 0 in bf16, small
    # enough to stay inside the HW ACT exp spline's defined input range
    # (exp of ~-1e8 returns NaN on real hardware, unlike the simulator).
    # With fp8 q/k the S accumulator carries a x4096 scale; so must the mask.
    negd = (
        np.eye(128, dtype=np.float32) * (-480.0 * (4096.0 if fp8qk else 1.0))
    ).astype(nbf16)
    tri = np.triu(np.ones((128, 128), dtype=np.float32)).astype(nbf16)
    f8 = ml_dtypes.float8_e4m3fn
    if mask_mode == "general":
        maskT = np.ascontiguousarray(np.asarray(mask).T.astype(np.float32)).astype(
            nbf16
        )

    in_maps = []
    for core in range(8):
        b, g = core // 2, core % 2
        hsl = slice(g * HL, (g + 1) * HL)
        # (h, d, e) -> (d, h*e)
        wq = np.ascontiguousarray(
            np.transpose(np.asarray(Wq)[hsl], (1, 0, 2)).reshape(D, 512)
        ).astype(nbf16)
        wk = np.ascontiguousarray(
            np.transpose(np.asarray(Wk)[hsl], (1, 0, 2)).reshape(D, 512)
        ).astype(nbf16)
        wv = np.ascontiguousarray(
            np.transpose(np.asarray(Wv)[hsl], (1, 0, 2)).reshape(D, 512)
        ).astype(nbf16)
        wo = np.ascontiguousarray(np.asarray(Wo)[:, g * 512 : (g + 1) * 512].T).astype(
            nbf16
        )
        xTb = np.ascontiguousarray(x[b].T)
        im = {
            "xT": xTb.astype(nbf16),
            "wv": wv,
            "wo": wo,
            "utri": utri,
            "negd": negd,
            "tri": tri,
        }
        if fp8qk:
            im["x8"] = np.clip(xTb, -240, 240).astype(f8)
            im["wq8"] = np.clip(
                np.transpose(np.asarray(Wq)[hsl], (1, 0, 2)).reshape(D, 512) * 64.0,
                -240,
                240,
            ).astype(f8)
            im["wk8"] = np.clip(
                np.transpose(np.asarray(Wk)[hsl], (1, 0, 2)).reshape(D, 512) * 64.0,
                -240,
                240,
            ).astype(f8)
        else:
            im["wq"] = wq
            im["wk"] = wk
        if mask_mode == "general":
            im["maskT"] = maskT
        if has_bias:
            im["wqb"] = np.asarray(bq)[hsl].reshape(1, 512).astype(nbf16)
            im["wkb"] = np.asarray(bk)[hsl].reshape(1, 512).astype(nbf16)
            im["wvb"] = np.asarray(bv)[hsl].reshape(1, 512).astype(nbf16)
        in_maps.append(im)

    global LAST_EXEC_TIME_NS, LAST_RESULTS
    if TRACE:
        _ensure_ntff_hook()
    res = run_bass_kernel_spmd(nc, in_maps, core_ids=list(range(8)), trace=TRACE)
    LAST_RESULTS = res
    if TRACE:
        LAST_EXEC_TIME_NS = res.exec_time_ns
    out = np.empty((B, T, D), dtype=np.float32)
    bo_f = np.asarray(bo, dtype=np.float32)
    for b in range(B):
        out[b] = res.results[2 * b]["y"] + res.results[2 * b + 1]["y"] + bo_f
    return out


# revision 69
# speedup vs baseline: 1.0143x; 1.0143x over previous
"""Multi-head causal attention (B=4, T=2048, D=1024, H=16, DH=64) on 8 trn2 cores.

Sharding: core = 2*b + g  (b = batch 0..3, g = head-group 0..1, 8 heads each).
Each core computes q/k/v projections for its 8 heads, causal attention, and the
row-parallel slice of the output projection; the host sums the two partial
outputs per batch and adds the output bias.

Dataflow (matmuls bf16 -> fp32 PSUM), designed so the PE never waits on the
softmax epilogue and the DVE never runs a serial reciprocal:

  xT (D,T) host pre-transposed, loaded as 8 per-kt tiles in 128KB DMA chunks
  qT/kT  [2-head pairs, 128 x T]  = Wpair.T @ x.T      (PE, K=128 d-tiles)
  v      [T-tiles 128 x 520]      = x @ Wv (+ ones col per head for row sums)
  ST     [j-tile 128, i-chunk 512] = kT.T @ qT          (K=64, 2 heads packed
                                     in row groups 0/1 -> concurrent MM pair)
  causal diagonal tiles: -480 strict-upper-triangle added INSIDE the S
     accumulation group via a [128x128] matmul (utri.T @ (-480 I)), so the
     exp output is already masked -- no DVE mask multiplies.  (-480, not
     -1e9: the HW ACT exp spline NaNs far outside its table range.)
  expST  = exp(ST/8)  (ScalarE, scale fused)
  av     [65, 512] += v_aug.T @ expST  (row 64 = softmax denominator l)
  epilogue: per head av[0:64] -> base-0 bf16 staging tile; the 4 l-rows of a
     half-chunk gather at partitions 0/32/64/96 of one tile; one
     reciprocal_approx_fast inverts all of them; each row spreads to a
     partition-0 tile (the GPSIMD broadcast ucode ignores AP partition
     offsets on HW), partition_broadcast -> [64,512] bf16, one DVE multiply
     per head writes concatT.  The division cluster is deferred into the
     next unit so its DVE backlog never gates PE-feeding evacuations.
  y      [T x 1024] = concatT.T @ WoT_g slices (K=128 c-tiles, fp32 out),
     evacuated per 512-col half and DMA'd straight to DRAM.

The v/q/k projections and the output projection are cut into single-matmul
"filler" thunks and drained into the attention loop between J iterations with
deadline tracking, so the PE stays busy (HAM stays warm) while ScalarE
crunches exp.  PSUM: spair 2x2 banks + av 2 + proj/outproj shared pool 2 = 8.
"""

import math
from collections import deque

import numpy as np
import ml_dtypes

import concourse.bass as bass
import concourse.bacc as bacc
import concourse.mybir as mybir
import concourse.tile as tile
from concourse.vector_clock import ScopedClock
from concourse.bass_utils import run_bass_kernel_spmd

BF16 = mybir.dt.bfloat16
F32 = mybir.dt.float32
nbf16 = ml_dtypes.bfloat16

D = 1024
DH = 64
HL = 8          # heads per core
KD = D // 128   # d-tiles


# ---------------------------------------------------------------------------
# Walrus in this build rejects >1 sync-wait on SP TPB_CTRL instructions; split
# the TileContext tail-drain's sem waits into single-wait SP nops.
def _patched_drain_and_barrier(self, tick_clock, wait_clock):
    nc = self.nc
    collector = nc.sync.nop()
    wait_clock.add_sem_waits(
        collector.ins, ScopedClock({None: tick_clock.global_clock})
    )
    si = collector.ins.sync_info
    waits = list(si.on_wait) if si and si.on_wait else []
    if si is not None:
        si.on_wait = waits[:1]
    for w in waits[1:]:
        extra = nc.sync.nop()
        esi = extra.ins.sync_info
        if esi is None:
            extra.ins.sync_info = mybir.SyncInfo(on_wait=[w], on_update=[])
        else:
            esi.on_wait = [w]
    nc.sync.drain()
    nc.all_engine_barrier()
    popped = nc._tile_sem_poison_stack.pop()
    assert popped is self._sem_poison
    nc.clear_and_free_semaphores(list(self.sems.allocated().values()))
    nc.all_engine_barrier()


def _apply_tile_patch():
    tile.TileContext._drain_and_barrier = _patched_drain_and_barrier


class _Filler:
    """FIFO of emission thunks with position markers for deadline drains."""

    def __init__(self):
        self.q = deque()
        self.added = 0
        self.drained = 0

    def add(self, thunks):
        self.q.extend(thunks)
        self.added += len(thunks)
        return self.added  # marker: drain_until(marker) runs through here

    def drain(self, k):
        k = min(k, len(self.q))
        for _ in range(k):
            self.q.popleft()()
        self.drained += k

    def drain_until(self, marker):
        while self.drained < marker and self.q:
            self.q.popleft()()
            self.drained += 1

    def drain_all(self):
        self.drain(len(self.q))


# ---------------------------------------------------------------------------
def build_core_program(T=2048, mask_mode="causal", has_bias=False):
    """One-core program; same NEFF runs SPMD on all 8 cores."""
    import os as _os

    MASKMM = _os.environ.get("K_MASKMM", "1") == "1"
    PIPE = _os.environ.get("K_PIPE", "1") == "1"
    FP8QK = _os.environ.get("K_FP8QK", "0") == "1" and not has_bias
    _apply_tile_patch()
    NT = T // 128            # 128-row t-tiles
    CH = min(512, T)         # i-chunk width
    NCH = T // CH            # chunks
    JT = CH // 128           # j-tiles per chunk

    FP8 = mybir.dt.float8e4
    nc = bacc.Bacc("TRN2", target_bir_lowering=False, debug=False)
    xT_d = nc.declare_dram_parameter("xT", [D, T], BF16, isOutput=False)
    if FP8QK:
        x8_d = nc.declare_dram_parameter("x8", [D, T], FP8, isOutput=False)
        wq8_d = nc.declare_dram_parameter("wq8", [D, 512], FP8, isOutput=False)
        wk8_d = nc.declare_dram_parameter("wk8", [D, 512], FP8, isOutput=False)
    else:
        wq_d = nc.declare_dram_parameter("wq", [D, 512], BF16, isOutput=False)
        wk_d = nc.declare_dram_parameter("wk", [D, 512], BF16, isOutput=False)
    wv_d = nc.declare_dram_parameter("wv", [D, 512], BF16, isOutput=False)
    wo_d = nc.declare_dram_parameter("wo", [512, D], BF16, isOutput=False)
    utri_d = nc.declare_dram_parameter("utri", [128, 128], BF16, isOutput=False)
    negd_d = nc.declare_dram_parameter("negd", [128, 128], BF16, isOutput=False)
    tri_d = nc.declare_dram_parameter("tri", [128, 128], BF16, isOutput=False)
    if mask_mode == "general":
        mt_d = nc.declare_dram_parameter("maskT", [T, T], BF16, isOutput=False)
    if has_bias:
        wqb_d = nc.declare_dram_parameter("wqb", [1, 512], BF16, isOutput=False)
        wkb_d = nc.declare_dram_parameter("wkb", [1, 512], BF16, isOutput=False)
        wvb_d = nc.declare_dram_parameter("wvb", [1, 512], BF16, isOutput=False)
    y_d = nc.declare_dram_parameter("y", [T, D], F32, isOutput=True)

    Exp = mybir.ActivationFunctionType.Exp
    Ln = mybir.ActivationFunctionType.Ln

    with tile.TileContext(nc) as tc:
        with (
            tc.tile_pool(name="singles", bufs=1) as singles,
            tc.tile_pool(name="est", bufs=6) as est_pool,
            tc.tile_pool(name="zst", bufs=10) as zpool,
            tc.tile_pool(name="lp", bufs=8) as lpool,
            tc.tile_pool(name="lbcp", bufs=6) as lbcp,
            tc.tile_pool(name="ysbp", bufs=4) as ysbp,
            tc.tile_pool(name="ps_big", bufs=2, space="PSUM") as ps_big,
            tc.tile_pool(name="ps_av", bufs=2, space="PSUM") as ps_av,
            tc.tile_pool(name="ps_px", bufs=2, space="PSUM") as ps_px,
        ):
            # ---- loads -------------------------------------------------
            # per-queue DMA is ~34GB/s, so gate-critical tensors are split
            # into 128KB chunks across queues: the first v matmul needs only
            # wv[kt0] + xT[kt0], ready ~4us in.
            wv_sb = singles.tile([128, KD, 512], BF16, name="wv_sb")
            wv_r = wv_d[:, :].rearrange("(kt p) n -> p kt n", p=128)
            xT_r = xT_d[:, :].rearrange("(kt p) t -> p kt t", p=128)
            if not FP8QK:
                wq_sb = singles.tile([128, KD, 512], BF16, name="wq_sb")
                wq_r = wq_d[:, :].rearrange("(kt p) n -> p kt n", p=128)
                wk_sb = singles.tile([128, KD, 512], BF16, name="wk_sb")
                wk_r = wk_d[:, :].rearrange("(kt p) n -> p kt n", p=128)
            # xT as KD x 4 independent [128, 512] column-chunk tiles: the
            # prologue (v tiles 0-3 + q/k of chunk 0) touches only column
            # chunk 0, so the 16 gate-critical DMAs (wv per-kt + xT cc0
            # per-kt) land across all 16 queues in ~4us instead of the PE
            # stalling ~24us for the full 4MB.
            NCC = max(1, T // 512)
            xT_cc = [[None] * NCC for _ in range(KD)]
            # gate-critical first: wv + xT column-chunk 0 pairwise (16 DMAs
            # across the 16 queues), then q/k weights (prologue qk pieces),
            # then the remaining xT column chunks.
            for kt in range(KD):
                nc.sync.dma_start(
                    out=wv_sb[:, kt : kt + 1, :], in_=wv_r[:, kt : kt + 1, :]
                )
                xk = singles.tile([128, 512], BF16, name=f"xT{kt}_0")
                nc.sync.dma_start(out=xk, in_=xT_r[:, kt, 0:512])
                xT_cc[kt][0] = xk
            if not FP8QK:
                for kt2 in range(KD):
                    nc.sync.dma_start(
                        out=wq_sb[:, kt2 : kt2 + 1, :],
                        in_=wq_r[:, kt2 : kt2 + 1, :],
                    )
                    nc.sync.dma_start(
                        out=wk_sb[:, kt2 : kt2 + 1, :],
                        in_=wk_r[:, kt2 : kt2 + 1, :],
                    )
            for cc in range(1, NCC):
                for kt in range(KD):
                    xk = singles.tile([128, 512], BF16, name=f"xT{kt}_{cc}")
                    nc.sync.dma_start(
                        out=xk, in_=xT_r[:, kt, cc * 512 : (cc + 1) * 512]
                    )
                    xT_cc[kt][cc] = xk

            class _XT:
                """xT_sb[:, kt, sl] view shim over the column-chunk tiles."""

                def __getitem__(self, key):
                    p_sl, kt, t_sl = key
                    cc = t_sl.start // 512
                    assert t_sl.stop <= (cc + 1) * 512, (t_sl, cc)
                    return xT_cc[kt][cc][
                        p_sl, t_sl.start - cc * 512 : t_sl.stop - cc * 512
                    ]

            xT_sb = _XT()
            if FP8QK:
                x8_sb = singles.tile([128, KD, T], FP8, name="x8_sb")
                x8_r = x8_d[:, :].rearrange("(kt p) t -> p kt t", p=128)
                for q8 in range(8):
                    cw = T // 8
                    nc.sync.dma_start(
                        out=x8_sb[:, :, q8 * cw : (q8 + 1) * cw],
                        in_=x8_r[:, :, q8 * cw : (q8 + 1) * cw],
                    )
                wq8_sb = singles.tile([128, KD, 512], FP8, name="wq8_sb")
                nc.sync.dma_start(
                    out=wq8_sb,
                    in_=wq8_d[:, :].rearrange("(kt p) n -> p kt n", p=128),
                )
                wk8_sb = singles.tile([128, KD, 512], FP8, name="wk8_sb")
                nc.sync.dma_start(
                    out=wk8_sb,
                    in_=wk8_d[:, :].rearrange("(kt p) n -> p kt n", p=128),
                )
            wo_sb = singles.tile([128, 4, D], BF16, name="wo_sb")
            nc.sync.dma_start(
                out=wo_sb, in_=wo_d[:, :].rearrange("(ct p) o -> p ct o", p=128)
            )
            utri_sb = singles.tile([128, 128], BF16, name="utri_sb")
            nc.sync.dma_start(out=utri_sb, in_=utri_d[:, :])
            negd_sb = singles.tile([128, 128], BF16, name="negd_sb")
            nc.sync.dma_start(out=negd_sb, in_=negd_d[:, :])
            tri_sb = singles.tile([128, 128], BF16, name="tri_sb")
            nc.sync.dma_start(out=tri_sb, in_=tri_d[:, :])
            if has_bias:
                wqb_sb = singles.tile([1, 512], BF16, name="wqb_sb")
                nc.sync.dma_start(out=wqb_sb, in_=wqb_d[:, :])
                wkb_sb = singles.tile([1, 512], BF16, name="wkb_sb")
                nc.sync.dma_start(out=wkb_sb, in_=wkb_d[:, :])
                wvb_sb = singles.tile([1, 512], BF16, name="wvb_sb")
                nc.sync.dma_start(out=wvb_sb, in_=wvb_d[:, :])
                ones_sb = singles.tile([1, T], BF16, name="ones_sb")
                nc.vector.memset(ones_sb, 1.0)

            ones128 = singles.tile([1, 128], BF16, name="ones128")
            nc.vector.memset(ones128, 1.0)
            v_sb = singles.tile([128, NT, 8 * 65], BF16, name="v_sb")
            # ones columns for all tiles/heads in one strided memset
            v_all = v_sb[:, :, :].rearrange("p t (h x) -> p t h x", x=65)
            nc.vector.memset(v_all[:, :, :, 64:65], 1.0)

            qT_sb = singles.tile([128, 4, T], BF16, name="qT_sb")
            kT_sb = singles.tile([128, 4, T], BF16, name="kT_sb")
            concat_sb = singles.tile([128, 4, T], BF16, name="concat_sb")

            # ---- projection / outproj pieces (filler thunks) -----------
            def v_piece(tt):
                st = {}

                def mk(kt):
                    def f():
                        if kt == 0:
                            st[0] = ps_px.tile(
                                [128, 512], F32, name="proj_ps", tag="px"
                            )
                        nc.tensor.matmul(
                            st[0],
                            xT_sb[:, kt, tt * 128 : (tt + 1) * 128],
                            wv_sb[:, kt, :],
                            start=(kt == 0),
                            stop=(kt == KD - 1 and not has_bias),
                        )
                    return f

                thunks = [mk(kt) for kt in range(KD)]
                if has_bias:
                    def fb():
                        nc.tensor.matmul(
                            st[0],
                            ones_sb[0:1, tt * 128 : (tt + 1) * 128],
                            wvb_sb[0:1, :],
                            start=False,
                            stop=True,
                        )
                    thunks.append(fb)

                def evac():
                    v_view = v_sb[:, tt, :].rearrange("p (h x) -> p h x", x=65)
                    nc.vector.tensor_copy(
                        v_view[:, :, 0:64],
                        st[0][:, 0:512].rearrange("p (h x) -> p h x", x=64),
                    )
                thunks.append(evac)
                return thunks

            def qk_piece(kind, c, pr):
                dst = qT_sb if kind == "q" else kT_sb
                cs = slice(c * CH, (c + 1) * CH)
                st = {}

                if FP8QK:
                    w8 = wq8_sb if kind == "q" else wk8_sb

                    def mk8(j):
                        def f():
                            if j == 0:
                                st[0] = ps_px.tile(
                                    [128, 512], F32, name="proj_ps", tag="px"
                                )
                            # fp8 DoubleRow: contraction pairs (2j, 2j+1)
                            # k-tiles on the same partition -> K=256 per MM
                            nc.tensor.matmul(
                                st[0][:, 0:CH],
                                w8[:, 2 * j : 2 * j + 2, pr * 128 : (pr + 1) * 128],
                                x8_sb[:, 2 * j : 2 * j + 2, cs],
                                start=(j == 0),
                                stop=(j == KD // 2 - 1),
                                perf_mode=mybir.MatmulPerfMode.DoubleRow,
                            )
                        return f

                    thunks = [mk8(j) for j in range(KD // 2)]

                    def evac():
                        nc.vector.tensor_copy(dst[:, pr, cs], st[0][:, 0:CH])
                    thunks.append(evac)
                    return thunks

                w_sb = wq_sb if kind == "q" else wk_sb

                def mk(kt):
                    def f():
                        if kt == 0:
                            st[0] = ps_px.tile(
                                [128, 512], F32, name="proj_ps", tag="px"
                            )
                        nc.tensor.matmul(
                            st[0][:, 0:CH],
                            w_sb[:, kt, pr * 128 : (pr + 1) * 128],
                            xT_sb[:, kt, cs],
                            start=(kt == 0),
                            stop=(kt == KD - 1 and not has_bias),
                        )
                    return f

                thunks = [mk(kt) for kt in range(KD)]
                if has_bias:
                    b_sb = wqb_sb if kind == "q" else wkb_sb

                    def fb():
                        nc.tensor.matmul(
                            st[0][:, 0:CH],
                            b_sb[0:1, pr * 128 : (pr + 1) * 128],
                            ones_sb[0:1, cs],
                            start=False,
                            stop=True,
                        )
                    thunks.append(fb)

                def evac():
                    nc.vector.tensor_copy(dst[:, pr, cs], st[0][:, 0:CH])
                thunks.append(evac)
                return thunks

            def outproj_piece(it):
                # ct-outer order: the two oc matmuls of each ct share the
                # same stationary operand (concat tile) back-to-back
                st = {}
                thunks = []

                def mk(ct, oc):
                    def f():
                        if ct == 0:
                            st[oc] = ps_px.tile(
                                [128, 512], F32, name="y_ps", tag="px"
                            )
                        nc.tensor.matmul(
                            st[oc],
                            concat_sb[:, ct, it * 128 : (it + 1) * 128],
                            wo_sb[:, ct, oc * 512 : (oc + 1) * 512],
                            start=(ct == 0),
                            stop=(ct == 3),
                        )
                    return f

                for ct in range(4):
                    thunks += [mk(ct, 0), mk(ct, 1)]

                def ev(oc):
                    def f():
                        y_sb = ysbp.tile([128, 512], F32, name="y_sb", tag="ysb")
                        nc.vector.tensor_copy(y_sb, st[oc])
                        nc.sync.dma_start(
                            out=y_d[
                                it * 128 : (it + 1) * 128,
                                oc * 512 : (oc + 1) * 512,
                            ],
                            in_=y_sb,
                        )
                    return f

                thunks += [ev(0), ev(1)]
                return thunks

            # ---- attention unit ----------------------------------------
            if mask_mode == "general":
                _mt_cm = tc.tile_pool(name="mtiles", bufs=NT + 2)
                mt_pool = _mt_cm.__enter__()

            def emit_unit(c, pr, filler, Lc, pending=None):
                cs = slice(c * CH, (c + 1) * CH)
                n_j = (c + 1) * JT if mask_mode == "causal" else NT
                if mask_mode == "general":
                    m_tiles = []
                    for J in range(n_j):
                        mt = mt_pool.tile([128, 512], BF16, name="mt", tag="mt")
                        nc.sync.dma_start(
                            out=mt[:, :CH], in_=mt_d[J * 128 : (J + 1) * 128, cs]
                        )
                        m_tiles.append(mt)
                av_t = [
                    ps_av.tile([65, 512], F32, name="av", tag="av")
                    for _ in range(2)
                ]
                s_tiles = {}

                def emit_S(J):
                    r = J - c * JT
                    diag = MASKMM and mask_mode == "causal" and 0 <= r < JT
                    off = max(0, r) * 128 if mask_mode == "causal" else 0
                    w = CH - off
                    spair = ps_big.tile([128, 1024], F32, name="spair", tag="big")
                    # head A at [off, CH); head B packed at [512, 512+w) so
                    # the exp range [off, 512+w) is gap-free.  Emit the two
                    # K=64 matmuls back-to-back: row groups 0/1 -> the PE
                    # runs them concurrently.
                    for hh in range(2):
                        hs = slice(hh * 64, (hh + 1) * 64)
                        dst = (
                            spair[:, off:CH] if hh == 0 else spair[:, 512 : 512 + w]
                        )
                        nc.tensor.matmul(
                            dst,
                            kT_sb[hs, pr, J * 128 : (J + 1) * 128],
                            qT_sb[hs, pr, c * CH + off : (c + 1) * CH],
                            start=True,
                            stop=not diag,
                            skip_group_check=diag,
                        )
                    if diag:
                        # accumulate -480 on the strict upper triangle of the
                        # diagonal 128x128 square: out[m,n] += -480*utri[n,m]
                        for hh in range(2):
                            d0 = off if hh == 0 else 512
                            nc.tensor.matmul(
                                spair[:, d0 : d0 + 128],
                                utri_sb,
                                negd_sb,
                                start=False,
                                stop=True,
                                skip_group_check=True,
                            )
                    s_tiles[J] = (spair, off)

                if PIPE:
                    pace = max(1, math.ceil(len(filler.q) / max(1, n_j)))
                else:
                    filler.drain_all()
                    pace = 0
                emit_S(0)
                for J in range(n_j):
                    filler.drain(pace)
                    if J + 1 < n_j:
                        emit_S(J + 1)
                    spair, off = s_tiles.pop(J)
                    w = CH - off
                    b_sl = [slice(off, CH), slice(512, 512 + w)]
                    e_pair = est_pool.tile([128, 1024], BF16, name="e_t", tag="e")
                    # with fp8 q/k the weights carry a x64 scale each, so S
                    # arrives x4096; fold the exact 2^-12 descale into exp
                    nc.scalar.activation(
                        e_pair[:, off : 512 + w],
                        spair[:, off : 512 + w],
                        Exp,
                        scale=0.125 / 4096.0 if FP8QK else 0.125,
                    )
                    r = J - c * JT
                    if (
                        not MASKMM
                        and mask_mode == "causal"
                        and 0 <= r < JT
                    ):
                        for hh in range(2):
                            d0 = b_sl[hh].start
                            nc.vector.tensor_mul(
                                e_pair[:, d0 : d0 + 128],
                                e_pair[:, d0 : d0 + 128],
                                tri_sb,
                            )
                    if mask_mode == "general":
                        for hh in range(2):
                            nc.vector.tensor_mul(
                                e_pair[:, b_sl[hh]],
                                e_pair[:, b_sl[hh]],
                                m_tiles[J][:, :CH],
                            )
                    for hh in range(2):
                        h = 2 * pr + hh
                        nc.tensor.matmul(
                            av_t[hh][:, off:CH],
                            v_sb[:, J, h * 65 : (h + 1) * 65],
                            e_pair[:, b_sl[hh]],
                            start=(J == 0),
                            stop=(J == n_j - 1),
                        )
                # epilogue: evacuate each head to a base-0 staging tile and
                # the two l rows into the half-chunk gather tile at
                # quadrant-aligned partitions (engine writes must start at
                # partition 0/32/64/96).
                zs = []
                # high priority: these copies free the av PSUM pair that the
                # NEXT unit's first AV matmul WAR-waits on; jump them ahead
                # of the division cluster in the DVE queue
                with tc.high_priority():
                    for hh in range(2):
                        z = zpool.tile([64, 512], BF16, name="z", tag="z")
                        nc.vector.tensor_copy(z[:, :CH], av_t[hh][0:64, :CH])
                        zs.append(z)
                        p0 = 64 * (pr % 2) + 32 * hh
                        nc.vector.tensor_copy(
                            Lc[p0 : p0 + 1, :CH], av_t[hh][64:65, :CH]
                        )
                if pending is not None:
                    # previous half-chunk's softmax-division cluster: emitted
                    # after this unit's drains and PSUM-freeing copies so its
                    # DVE backlog gates as little PE work as possible
                    pending()
                if mask_mode == "general":
                    del m_tiles
                return zs

            # ---- schedule ----------------------------------------------
            filler = _Filler()
            units = [(c, pr) for c in range(NCH) for pr in range(4)]

            # prologue: v tiles for chunk 0 (all tiles unless causal), q/k for
            # the first two units
            n_v_pro = JT if mask_mode == "causal" else NT
            for tt in range(n_v_pro):
                for th in v_piece(tt):
                    th()
            for c, pr in units[: min(2, len(units))]:
                for th in qk_piece("q", c, pr):
                    th()
                for th in qk_piece("k", c, pr):
                    th()

            qk_markers = {}
            v_markers = {}
            qk_stream = units[2:]
            v_next = n_v_pro  # next v tile to enqueue

            def make_division(c, z_pair, Lc):
                """Closure emitting the half-chunk softmax-division cluster
                (on DVE; the ACT Ln/Exp route flip-flops activation table
                sets with the attention Exp -- 1.3us reload each time).  The
                4 l rows sit at partitions 0/32/64/96; in-between lanes hold
                1.0 and are never read."""
                prs = list(z_pair.keys())
                # the very last division runs at drain time with ScalarE
                # still crunching the final unit's exp backlog -- spread the
                # Linv rows on the (then-idle) DVE instead so the tail
                # doesn't wait on the ACT queue
                spread_dve = c == NCH - 1 and prs[-1] == 3

                def emit():
                    Linv = lpool.tile([128, 512], F32, name="Linv", tag="linv")
                    # custom-DVE bit-trick reciprocal, ~5x faster than the
                    # iterative divide; HW-verified on this exact [0:97] AP
                    nc.vector.reciprocal_approx_fast(
                        Linv[0:97, :CH], Lc[0:97, :CH]
                    )
                    cs = slice(c * CH, (c + 1) * CH)
                    for pr2 in prs:
                        for hh in range(2):
                            p0 = 64 * (pr2 % 2) + 32 * hh
                            # the GPSIMD broadcast ucode ignores AP partition
                            # offsets on HW (reads p0, writes from p0), so
                            # spread each Linv row to a partition-0 tile
                            # first; row 0 can be read in place.  The spreads
                            # run on ScalarE (its copy shares the exp table
                            # set) to keep the DVE queue short here -- these
                            # only gate the deferred broadcasts.
                            li = lpool.tile([1, 512], BF16, name="li", tag="li")
                            if spread_dve:
                                nc.vector.tensor_copy(
                                    li[0:1, :CH], Linv[p0 : p0 + 1, :CH]
                                )
                            else:
                                nc.scalar.copy(
                                    li[0:1, :CH], Linv[p0 : p0 + 1, :CH]
                                )
                            li_ap = li[0:1, :CH]
                            hs = slice(hh * 64, (hh + 1) * 64)
                            if spread_dve:
                                # drain time: the PE is idle and ps_px free;
                                # broadcast via a K=1 ones-matmul (213ns)
                                # instead of 4 serial 1.1us GPSIMD broadcasts
                                lbc_ps = ps_px.tile(
                                    [128, 512], F32, name="lbc_ps", tag="px"
                                )
                                nc.tensor.matmul(
                                    lbc_ps[0:64, :CH],
                                    ones128[0:1, 0:64],
                                    li_ap,
                                    start=True,
                                    stop=True,
                                )
                                nc.vector.tensor_mul(
                                    concat_sb[hs, pr2, cs],
                                    z_pair[pr2][hh][:, :CH],
                                    lbc_ps[0:64, :CH],
                                )
                            else:
                                lbc = lbcp.tile(
                                    [64, 512], BF16, name="lbc", tag="lbc"
                                )
                                nc.gpsimd.partition_broadcast(
                                    lbc[:, :CH], li_ap, channels=64
                                )
                                nc.vector.tensor_mul(
                                    concat_sb[hs, pr2, cs],
                                    z_pair[pr2][hh][:, :CH],
                                    lbc[:, :CH],
                                )

                return emit

            pending_div = None
            for u, (c, pr) in enumerate(units):
                # enqueue filler due soon (outproj last so its matmuls drain
                # late in the J loop, after the previous half-chunk's
                # division cluster has emitted its concat multiplies)
                if u < len(qk_stream):
                    c2, pr2 = qk_stream[u]
                    m = filler.add(qk_piece("q", c2, pr2))
                    m = filler.add(qk_piece("k", c2, pr2))
                    qk_markers[(c2, pr2)] = m
                if v_next < NT and u >= 1:
                    # enqueue one unit later than strictly possible: the
                    # cc>=1 xT chunks land ~40-50us in, and an eagerly
                    # drained v matmul at the PE queue head would stall
                    # everything behind it on the DMA
                    m = filler.add(v_piece(v_next))
                    v_markers[v_next] = m
                    v_next += 1
                # outproj tile u-5: shifted one unit past the (c-1, pr-1)
                # ready point so its matmuls are always emitted after the
                # (deferred) division cluster that writes its concat inputs
                if 0 <= u - 5 < (NCH - 1) * JT:
                    filler.add(outproj_piece(u - 5))
                if u == len(units) - 1 and (NCH - 1) * JT - 1 >= 0:
                    # last unit also drains the final previous-chunk tile
                    filler.add(outproj_piece((NCH - 1) * JT - 1))

                # deadlines: q/k of this unit, v tiles of this chunk
                if (c, pr) in qk_markers:
                    filler.drain_until(qk_markers[(c, pr)])
                vt_needed = (c + 1) * JT - 1 if mask_mode == "causal" else NT - 1
                if vt_needed in v_markers:
                    filler.drain_until(v_markers[vt_needed])

                if pr % 2 == 0:
                    Lc = lpool.tile([128, 512], F32, name="Lc", tag="lc")
                    # initialize so the reciprocal over [0:97] never sees
                    # garbage; same queue as the l-copies so ordering is by
                    # emission
                    nc.vector.memset(Lc, 1.0)
                    z_pair = {}
                z_pair[pr] = emit_unit(c, pr, filler, Lc, pending=pending_div)
                pending_div = None

                if pr % 2 == 1:
                    pending_div = make_division(c, z_pair, Lc)

            # drain: final division, remaining filler, last outproj tiles
            if pending_div is not None:
                pending_div()
            filler.drain_all()
            for it in range((NCH - 1) * JT, NCH * JT):
                for th in outproj_piece(it):
                    th()
            if mask_mode == "general":
                _mt_cm.__exit__(None, None, None)
    nc.finalize()
    return nc


# ---------------------------------------------------------------------------
# Optional NTFF profiling (test.py sets TRACE=True). Registers the missing
# antenv.axon_hooks module so run_bass_kernel_spmd's trace path works.
TRACE = False
LAST_EXEC_TIME_NS = None
LAST_RESULTS = None


def _ensure_ntff_hook():
    import sys as _sys
    import types as _types

    if "antenv.axon_hooks" in _sys.modules:
        return
    mod = _types.ModuleType("antenv.axon_hooks")
    state = {"hook": None}
    mod.set_axon_ntff_profile_hook = lambda h: state.__setitem__("hook", h)
    mod.get_axon_ntff_profile_hook = lambda: state["hook"]
    _sys.modules["antenv.axon_hooks"] = mod
    import antenv

    antenv.axon_hooks = mod
    try:
        from trn_agent_boot.trn_boot import _ntff_profile_via_ctypes

        hook = _ntff_profile_via_ctypes("/opt/axon/libaxon_pjrt.so")
        if hook is not None:
            mod.set_axon_ntff_profile_hook(hook)
    except Exception:
        pass


_PROGRAM_CACHE = {}


def _get_program(T, mask_mode, has_bias):
    key = (T, mask_mode, has_bias)
    if key not in _PROGRAM_CACHE:
        _PROGRAM_CACHE[key] = build_core_program(T, mask_mode, has_bias)
    return _PROGRAM_CACHE[key]


def _mask_mode_of(mask):
    m = np.asarray(mask)
    if m.all():
        return "full"
    T = m.shape[0]
    tril = np.tril(np.ones((T, T), dtype=bool))
    if np.array_equal(m.astype(bool), tril):
        return "causal"
    return "general"


def kernel(x, mask, Wq, bq, Wk, bk, Wv, bv, Wo, bo):
    x = np.asarray(x)
    B, T, D_ = x.shape
    H = Wq.shape[0]
    assert D_ == D and H == 16
    mask_mode = _mask_mode_of(mask)
    has_bias = bool(
        np.any(np.asarray(bq)) or np.any(np.asarray(bk)) or np.any(np.asarray(bv))
    )
    nc = _get_program(T, mask_mode, has_bias)

    import os as _os

    fp8qk = _os.environ.get("K_FP8QK", "0") == "1" and not has_bias
    utri = np.triu(np.ones((128, 128), dtype=np.float32), 1).astype(nbf16)
    # -480: large enough that exp((S-480)/8) ~ 4e-26 ~ 0 in bf16, small
    # enough to stay inside the HW ACT exp spline's defined input range
    # (exp of ~-1e8 returns NaN on real hardware, unlike the simulator).
    # With fp8 q/k the S accumulator carries a x4096 scale; so must the mask.
    negd = (
        np.eye(128, dtype=np.float32) * (-480.0 * (4096.0 if fp8qk else 1.0))
    ).astype(nbf16)
    tri = np.triu(np.ones((128, 128), dtype=np.float32)).astype(nbf16)
    f8 = ml_dtypes.float8_e4m3fn
    if mask_mode == "general":
        maskT = np.ascontiguousarray(np.asarray(mask).T.astype(np.float32)).astype(
            nbf16
        )

    in_maps = []
    for core in range(8):
        b, g = core // 2, core % 2
        hsl = slice(g * HL, (g + 1) * HL)
        # (h, d, e) -> (d, h*e)
        wq = np.ascontiguousarray(
            np.transpose(np.asarray(Wq)[hsl], (1, 0, 2)).reshape(D, 512)
        ).astype(nbf16)
        wk = np.ascontiguousarray(
            np.transpose(np.asarray(Wk)[hsl], (1, 0, 2)).reshape(D, 512)
        ).astype(nbf16)
        wv = np.ascontiguousarray(
            np.transpose(np.asarray(Wv)[hsl], (1, 0, 2)).reshape(D, 512)
        ).astype(nbf16)
        wo = np.ascontiguousarray(np.asarray(Wo)[:, g * 512 : (g + 1) * 512].T).astype(
            nbf16
        )
        xTb = np.ascontiguousarray(x[b].T)
        im = {
            "xT": xTb.astype(nbf16),
            "wv": wv,
            "wo": wo,
            "utri": utri,
            "negd": negd,
            "tri": tri,
        }
        if fp8qk:
            im["x8"] = np.clip(xTb, -240, 240).astype(f8)
            im["wq8"] = np.clip(
                np.transpose(np.asarray(Wq)[hsl], (1, 0, 2)).reshape(D, 512) * 64.0,
                -240,
                240,
            ).astype(f8)
            im["wk8"] = np.clip(
                np.transpose(np.asarray(Wk)[hsl], (1, 0, 2)).reshape(D, 512) * 64.0,
                -240,
                240,
            ).astype(f8)
        else:
            im["wq"] = wq
            im["wk"] = wk
        if mask_mode == "general":
            im["maskT"] = maskT
        if has_bias:
            im["wqb"] = np.asarray(bq)[hsl].reshape(1, 512).astype(nbf16)
            im["wkb"] = np.asarray(bk)[hsl].reshape(1, 512).astype(nbf16)
            im["wvb"] = np.asarray(bv)[hsl].reshape(1, 512).astype(nbf16)
        in_maps.append(im)

    global LAST_EXEC_TIME_NS, LAST_RESULTS
    if TRACE:
        _ensure_ntff_hook()
    res = run_bass_kernel_spmd(nc, in_maps, core_ids=list(range(8)), trace=TRACE)
    LAST_RESULTS = res
    if TRACE:
        LAST_EXEC_TIME_NS = res.exec_time_ns
    out = np.empty((B, T, D), dtype=np.float32)
    bo_f = np.asarray(bo, dtype=np.float32)
    for b in range(B):
        out[b] = res.results[2 * b]["y"] + res.results[2 * b + 1]["y"] + bo_f
    return out


# revision 70
# speedup vs baseline: 1.0254x; 1.0110x over previous
"""Multi-head causal attention (B=4, T=2048, D=1024, H=16, DH=64) on 8 trn2 cores.

Sharding: core = 2*b + g  (b = batch 0..3, g = head-group 0..1, 8 heads each).
Each core computes q/k/v projections for its 8 heads, causal attention, and the
row-parallel slice of the output projection; the host sums the two partial
outputs per batch and adds the output bias.

Dataflow (matmuls bf16 -> fp32 PSUM), designed so the PE never waits on the
softmax epilogue and the DVE never runs a serial reciprocal:

  xT (D,T) host pre-transposed, loaded as 8 per-kt tiles in 128KB DMA chunks
  qT/kT  [2-head pairs, 128 x T]  = Wpair.T @ x.T      (PE, K=128 d-tiles)
  v      [T-tiles 128 x 520]      = x @ Wv (+ ones col per head for row sums)
  ST     [j-tile 128, i-chunk 512] = kT.T @ qT          (K=64, 2 heads packed
                                     in row groups 0/1 -> concurrent MM pair)
  causal diagonal tiles: -480 strict-upper-triangle added INSIDE the S
     accumulation group via a [128x128] matmul (utri.T @ (-480 I)), so the
     exp output is already masked -- no DVE mask multiplies.  (-480, not
     -1e9: the HW ACT exp spline NaNs far outside its table range.)
  expST  = exp(ST/8)  (ScalarE, scale fused)
  av     [65, 512] += v_aug.T @ expST  (row 64 = softmax denominator l)
  epilogue: per head av[0:64] -> base-0 bf16 staging tile; the 4 l-rows of a
     half-chunk gather at partitions 0/32/64/96 of one tile; one
     reciprocal_approx_fast inverts all of them; each row spreads to a
     partition-0 tile (the GPSIMD broadcast ucode ignores AP partition
     offsets on HW), partition_broadcast -> [64,512] bf16, one DVE multiply
     per head writes concatT.  The division cluster is deferred into the
     next unit so its DVE backlog never gates PE-feeding evacuations.
  y      [T x 1024] = concatT.T @ WoT_g slices (K=128 c-tiles, fp32 out),
     evacuated per 512-col half and DMA'd straight to DRAM.

The v/q/k projections and the output projection are cut into single-matmul
"filler" thunks and drained into the attention loop between J iterations with
deadline tracking, so the PE stays busy (HAM stays warm) while ScalarE
crunches exp.  PSUM: spair 2x2 banks + av 2 + proj/outproj shared pool 2 = 8.
"""

import math
from collections import deque

import numpy as np
import ml_dtypes

import concourse.bass as bass
import concourse.bacc as bacc
import concourse.mybir as mybir
import concourse.tile as tile
from concourse.vector_clock import ScopedClock
from concourse.bass_utils import run_bass_kernel_spmd

BF16 = mybir.dt.bfloat16
F32 = mybir.dt.float32
nbf16 = ml_dtypes.bfloat16

D = 1024
DH = 64
HL = 8          # heads per core
KD = D // 128   # d-tiles


# ---------------------------------------------------------------------------
# Walrus in this build rejects >1 sync-wait on SP TPB_CTRL instructions; split
# the TileContext tail-drain's sem waits into single-wait SP nops.
def _patched_drain_and_barrier(self, tick_clock, wait_clock):
    nc = self.nc
    collector = nc.sync.nop()
    wait_clock.add_sem_waits(
        collector.ins, ScopedClock({None: tick_clock.global_clock})
    )
    si = collector.ins.sync_info
    waits = list(si.on_wait) if si and si.on_wait else []
    if si is not None:
        si.on_wait = waits[:1]
    for w in waits[1:]:
        extra = nc.sync.nop()
        esi = extra.ins.sync_info
        if esi is None:
            extra.ins.sync_info = mybir.SyncInfo(on_wait=[w], on_update=[])
        else:
            esi.on_wait = [w]
    nc.sync.drain()
    nc.all_engine_barrier()
    popped = nc._tile_sem_poison_stack.pop()
    assert popped is self._sem_poison
    nc.clear_and_free_semaphores(list(self.sems.allocated().values()))
    nc.all_engine_barrier()


def _apply_tile_patch():
    tile.TileContext._drain_and_barrier = _patched_drain_and_barrier


class _Filler:
    """FIFO of emission thunks with position markers for deadline drains."""

    def __init__(self):
        self.q = deque()
        self.added = 0
        self.drained = 0

    def add(self, thunks):
        self.q.extend(thunks)
        self.added += len(thunks)
        return self.added  # marker: drain_until(marker) runs through here

    def drain(self, k):
        k = min(k, len(self.q))
        for _ in range(k):
            self.q.popleft()()
        self.drained += k

    def drain_until(self, marker):
        while self.drained < marker and self.q:
            self.q.popleft()()
            self.drained += 1

    def drain_all(self):
        self.drain(len(self.q))


# ---------------------------------------------------------------------------
def build_core_program(T=2048, mask_mode="causal", has_bias=False):
    """One-core program; same NEFF runs SPMD on all 8 cores."""
    import os as _os

    MASKMM = _os.environ.get("K_MASKMM", "1") == "1"
    PIPE = _os.environ.get("K_PIPE", "1") == "1"
    FP8QK = _os.environ.get("K_FP8QK", "0") == "1" and not has_bias
    _apply_tile_patch()
    NT = T // 128            # 128-row t-tiles
    CH = min(512, T)         # i-chunk width
    NCH = T // CH            # chunks
    JT = CH // 128           # j-tiles per chunk

    FP8 = mybir.dt.float8e4
    nc = bacc.Bacc("TRN2", target_bir_lowering=False, debug=False)
    xT_d = nc.declare_dram_parameter("xT", [D, T], BF16, isOutput=False)
    if FP8QK:
        x8_d = nc.declare_dram_parameter("x8", [D, T], FP8, isOutput=False)
        wq8_d = nc.declare_dram_parameter("wq8", [D, 512], FP8, isOutput=False)
        wk8_d = nc.declare_dram_parameter("wk8", [D, 512], FP8, isOutput=False)
    else:
        wq_d = nc.declare_dram_parameter("wq", [D, 512], BF16, isOutput=False)
        wk_d = nc.declare_dram_parameter("wk", [D, 512], BF16, isOutput=False)
    wv_d = nc.declare_dram_parameter("wv", [D, 512], BF16, isOutput=False)
    wo_d = nc.declare_dram_parameter("wo", [512, D], BF16, isOutput=False)
    utri_d = nc.declare_dram_parameter("utri", [128, 128], BF16, isOutput=False)
    negd_d = nc.declare_dram_parameter("negd", [128, 128], BF16, isOutput=False)
    tri_d = nc.declare_dram_parameter("tri", [128, 128], BF16, isOutput=False)
    if mask_mode == "general":
        mt_d = nc.declare_dram_parameter("maskT", [T, T], BF16, isOutput=False)
    if has_bias:
        wqb_d = nc.declare_dram_parameter("wqb", [1, 512], BF16, isOutput=False)
        wkb_d = nc.declare_dram_parameter("wkb", [1, 512], BF16, isOutput=False)
        wvb_d = nc.declare_dram_parameter("wvb", [1, 512], BF16, isOutput=False)
    y_d = nc.declare_dram_parameter("y", [T, D], F32, isOutput=True)

    Exp = mybir.ActivationFunctionType.Exp
    Ln = mybir.ActivationFunctionType.Ln

    with tile.TileContext(nc) as tc:
        with (
            tc.tile_pool(name="singles", bufs=1) as singles,
            tc.tile_pool(name="est", bufs=6) as est_pool,
            tc.tile_pool(name="zst", bufs=10) as zpool,
            tc.tile_pool(name="lp", bufs=8) as lpool,
            tc.tile_pool(name="lbcp", bufs=6) as lbcp,
            tc.tile_pool(name="ysbp", bufs=4) as ysbp,
            tc.tile_pool(name="ps_big", bufs=2, space="PSUM") as ps_big,
            tc.tile_pool(name="ps_av", bufs=2, space="PSUM") as ps_av,
            tc.tile_pool(name="ps_px", bufs=2, space="PSUM") as ps_px,
        ):
            # ---- loads -------------------------------------------------
            # per-queue DMA is ~34GB/s, so gate-critical tensors are split
            # into 128KB chunks across queues: the first v matmul needs only
            # wv[kt0] + xT[kt0], ready ~4us in.
            # mask constants first: chunk 0's every J-tile is diagonal, so
            # the first unit's mask matmuls need these 32KB tiles immediately
            utri_sb = singles.tile([128, 128], BF16, name="utri_sb")
            nc.sync.dma_start(out=utri_sb, in_=utri_d[:, :])
            negd_sb = singles.tile([128, 128], BF16, name="negd_sb")
            nc.sync.dma_start(out=negd_sb, in_=negd_d[:, :])
            tri_sb = singles.tile([128, 128], BF16, name="tri_sb")
            nc.sync.dma_start(out=tri_sb, in_=tri_d[:, :])
            wv_sb = singles.tile([128, KD, 512], BF16, name="wv_sb")
            wv_r = wv_d[:, :].rearrange("(kt p) n -> p kt n", p=128)
            xT_r = xT_d[:, :].rearrange("(kt p) t -> p kt t", p=128)
            if not FP8QK:
                wq_sb = singles.tile([128, KD, 512], BF16, name="wq_sb")
                wq_r = wq_d[:, :].rearrange("(kt p) n -> p kt n", p=128)
                wk_sb = singles.tile([128, KD, 512], BF16, name="wk_sb")
                wk_r = wk_d[:, :].rearrange("(kt p) n -> p kt n", p=128)
            # xT as KD x 4 independent [128, 512] column-chunk tiles: the
            # prologue (v tiles 0-3 + q/k of chunk 0) touches only column
            # chunk 0, so the 16 gate-critical DMAs (wv per-kt + xT cc0
            # per-kt) land across all 16 queues in ~4us instead of the PE
            # stalling ~24us for the full 4MB.
            NCC = max(1, T // 512)
            xT_cc = [[None] * NCC for _ in range(KD)]
            # gate-critical first: wv + xT column-chunk 0 pairwise (16 DMAs
            # across the 16 queues), then q/k weights (prologue qk pieces),
            # then the remaining xT column chunks.
            for kt in range(KD):
                nc.sync.dma_start(
                    out=wv_sb[:, kt : kt + 1, :], in_=wv_r[:, kt : kt + 1, :]
                )
                xk = singles.tile([128, 512], BF16, name=f"xT{kt}_0")
                nc.sync.dma_start(out=xk, in_=xT_r[:, kt, 0:512])
                xT_cc[kt][0] = xk
            if not FP8QK:
                for kt2 in range(KD):
                    nc.sync.dma_start(
                        out=wq_sb[:, kt2 : kt2 + 1, :],
                        in_=wq_r[:, kt2 : kt2 + 1, :],
                    )
                    nc.sync.dma_start(
                        out=wk_sb[:, kt2 : kt2 + 1, :],
                        in_=wk_r[:, kt2 : kt2 + 1, :],
                    )
            for cc in range(1, NCC):
                for kt in range(KD):
                    xk = singles.tile([128, 512], BF16, name=f"xT{kt}_{cc}")
                    nc.sync.dma_start(
                        out=xk, in_=xT_r[:, kt, cc * 512 : (cc + 1) * 512]
                    )
                    xT_cc[kt][cc] = xk

            class _XT:
                """xT_sb[:, kt, sl] view shim over the column-chunk tiles."""

                def __getitem__(self, key):
                    p_sl, kt, t_sl = key
                    cc = t_sl.start // 512
                    assert t_sl.stop <= (cc + 1) * 512, (t_sl, cc)
                    return xT_cc[kt][cc][
                        p_sl, t_sl.start - cc * 512 : t_sl.stop - cc * 512
                    ]

            xT_sb = _XT()
            if FP8QK:
                x8_sb = singles.tile([128, KD, T], FP8, name="x8_sb")
                x8_r = x8_d[:, :].rearrange("(kt p) t -> p kt t", p=128)
                for q8 in range(8):
                    cw = T // 8
                    nc.sync.dma_start(
                        out=x8_sb[:, :, q8 * cw : (q8 + 1) * cw],
                        in_=x8_r[:, :, q8 * cw : (q8 + 1) * cw],
                    )
                wq8_sb = singles.tile([128, KD, 512], FP8, name="wq8_sb")
                nc.sync.dma_start(
                    out=wq8_sb,
                    in_=wq8_d[:, :].rearrange("(kt p) n -> p kt n", p=128),
                )
                wk8_sb = singles.tile([128, KD, 512], FP8, name="wk8_sb")
                nc.sync.dma_start(
                    out=wk8_sb,
                    in_=wk8_d[:, :].rearrange("(kt p) n -> p kt n", p=128),
                )
            wo_sb = singles.tile([128, 4, D], BF16, name="wo_sb")
            nc.sync.dma_start(
                out=wo_sb, in_=wo_d[:, :].rearrange("(ct p) o -> p ct o", p=128)
            )
            if has_bias:
                wqb_sb = singles.tile([1, 512], BF16, name="wqb_sb")
                nc.sync.dma_start(out=wqb_sb, in_=wqb_d[:, :])
                wkb_sb = singles.tile([1, 512], BF16, name="wkb_sb")
                nc.sync.dma_start(out=wkb_sb, in_=wkb_d[:, :])
                wvb_sb = singles.tile([1, 512], BF16, name="wvb_sb")
                nc.sync.dma_start(out=wvb_sb, in_=wvb_d[:, :])
                ones_sb = singles.tile([1, T], BF16, name="ones_sb")
                nc.vector.memset(ones_sb, 1.0)

            ones128 = singles.tile([1, 128], BF16, name="ones128")
            nc.vector.memset(ones128, 1.0)
            v_sb = singles.tile([128, NT, 8 * 65], BF16, name="v_sb")
            # ones columns for all tiles/heads in one strided memset
            v_all = v_sb[:, :, :].rearrange("p t (h x) -> p t h x", x=65)
            nc.vector.memset(v_all[:, :, :, 64:65], 1.0)

            qT_sb = singles.tile([128, 4, T], BF16, name="qT_sb")
            kT_sb = singles.tile([128, 4, T], BF16, name="kT_sb")
            concat_sb = singles.tile([128, 4, T], BF16, name="concat_sb")

            # ---- projection / outproj pieces (filler thunks) -----------
            def v_piece(tt):
                st = {}

                def mk(kt):
                    def f():
                        if kt == 0:
                            st[0] = ps_px.tile(
                                [128, 512], F32, name="proj_ps", tag="px"
                            )
                        nc.tensor.matmul(
                            st[0],
                            xT_sb[:, kt, tt * 128 : (tt + 1) * 128],
                            wv_sb[:, kt, :],
                            start=(kt == 0),
                            stop=(kt == KD - 1 and not has_bias),
                        )
                    return f

                thunks = [mk(kt) for kt in range(KD)]
                if has_bias:
                    def fb():
                        nc.tensor.matmul(
                            st[0],
                            ones_sb[0:1, tt * 128 : (tt + 1) * 128],
                            wvb_sb[0:1, :],
                            start=False,
                            stop=True,
                        )
                    thunks.append(fb)

                def evac():
                    v_view = v_sb[:, tt, :].rearrange("p (h x) -> p h x", x=65)
                    nc.vector.tensor_copy(
                        v_view[:, :, 0:64],
                        st[0][:, 0:512].rearrange("p (h x) -> p h x", x=64),
                    )
                thunks.append(evac)
                return thunks

            def qk_piece(kind, c, pr):
                dst = qT_sb if kind == "q" else kT_sb
                cs = slice(c * CH, (c + 1) * CH)
                st = {}

                if FP8QK:
                    w8 = wq8_sb if kind == "q" else wk8_sb

                    def mk8(j):
                        def f():
                            if j == 0:
                                st[0] = ps_px.tile(
                                    [128, 512], F32, name="proj_ps", tag="px"
                                )
                            # fp8 DoubleRow: contraction pairs (2j, 2j+1)
                            # k-tiles on the same partition -> K=256 per MM
                            nc.tensor.matmul(
                                st[0][:, 0:CH],
                                w8[:, 2 * j : 2 * j + 2, pr * 128 : (pr + 1) * 128],
                                x8_sb[:, 2 * j : 2 * j + 2, cs],
                                start=(j == 0),
                                stop=(j == KD // 2 - 1),
                                perf_mode=mybir.MatmulPerfMode.DoubleRow,
                            )
                        return f

                    thunks = [mk8(j) for j in range(KD // 2)]

                    def evac():
                        nc.vector.tensor_copy(dst[:, pr, cs], st[0][:, 0:CH])
                    thunks.append(evac)
                    return thunks

                w_sb = wq_sb if kind == "q" else wk_sb

                def mk(kt):
                    def f():
                        if kt == 0:
                            st[0] = ps_px.tile(
                                [128, 512], F32, name="proj_ps", tag="px"
                            )
                        nc.tensor.matmul(
                            st[0][:, 0:CH],
                            w_sb[:, kt, pr * 128 : (pr + 1) * 128],
                            xT_sb[:, kt, cs],
                            start=(kt == 0),
                            stop=(kt == KD - 1 and not has_bias),
                        )
                    return f

                thunks = [mk(kt) for kt in range(KD)]
                if has_bias:
                    b_sb = wqb_sb if kind == "q" else wkb_sb

                    def fb():
                        nc.tensor.matmul(
                            st[0][:, 0:CH],
                            b_sb[0:1, pr * 128 : (pr + 1) * 128],
                            ones_sb[0:1, cs],
                            start=False,
                            stop=True,
                        )
                    thunks.append(fb)

                def evac():
                    nc.vector.tensor_copy(dst[:, pr, cs], st[0][:, 0:CH])
                thunks.append(evac)
                return thunks

            def outproj_piece(it):
                # ct-outer order: the two oc matmuls of each ct share the
                # same stationary operand (concat tile) back-to-back
                st = {}
                thunks = []

                def mk(ct, oc):
                    def f():
                        if ct == 0:
                            st[oc] = ps_px.tile(
                                [128, 512], F32, name="y_ps", tag="px"
                            )
                        nc.tensor.matmul(
                            st[oc],
                            concat_sb[:, ct, it * 128 : (it + 1) * 128],
                            wo_sb[:, ct, oc * 512 : (oc + 1) * 512],
                            start=(ct == 0),
                            stop=(ct == 3),
                        )
                    return f

                for ct in range(4):
                    thunks += [mk(ct, 0), mk(ct, 1)]

                def ev(oc):
                    def f():
                        y_sb = ysbp.tile([128, 512], F32, name="y_sb", tag="ysb")
                        nc.vector.tensor_copy(y_sb, st[oc])
                        nc.sync.dma_start(
                            out=y_d[
                                it * 128 : (it + 1) * 128,
                                oc * 512 : (oc + 1) * 512,
                            ],
                            in_=y_sb,
                        )
                    return f

                thunks += [ev(0), ev(1)]
                return thunks

            # ---- attention unit ----------------------------------------
            if mask_mode == "general":
                _mt_cm = tc.tile_pool(name="mtiles", bufs=NT + 2)
                mt_pool = _mt_cm.__enter__()

            def emit_unit(c, pr, filler, Lc, pending=None):
                cs = slice(c * CH, (c + 1) * CH)
                n_j = (c + 1) * JT if mask_mode == "causal" else NT
                if mask_mode == "general":
                    m_tiles = []
                    for J in range(n_j):
                        mt = mt_pool.tile([128, 512], BF16, name="mt", tag="mt")
                        nc.sync.dma_start(
                            out=mt[:, :CH], in_=mt_d[J * 128 : (J + 1) * 128, cs]
                        )
                        m_tiles.append(mt)
                av_t = [
                    ps_av.tile([65, 512], F32, name="av", tag="av")
                    for _ in range(2)
                ]
                s_tiles = {}

                def emit_S(J):
                    r = J - c * JT
                    diag = MASKMM and mask_mode == "causal" and 0 <= r < JT
                    off = max(0, r) * 128 if mask_mode == "causal" else 0
                    w = CH - off
                    spair = ps_big.tile([128, 1024], F32, name="spair", tag="big")
                    # head A at [off, CH); head B packed at [512, 512+w) so
                    # the exp range [off, 512+w) is gap-free.  Emit the two
                    # K=64 matmuls back-to-back: row groups 0/1 -> the PE
                    # runs them concurrently.
                    for hh in range(2):
                        hs = slice(hh * 64, (hh + 1) * 64)
                        dst = (
                            spair[:, off:CH] if hh == 0 else spair[:, 512 : 512 + w]
                        )
                        nc.tensor.matmul(
                            dst,
                            kT_sb[hs, pr, J * 128 : (J + 1) * 128],
                            qT_sb[hs, pr, c * CH + off : (c + 1) * CH],
                            start=True,
                            stop=not diag,
                            skip_group_check=diag,
                        )
                    if diag:
                        # accumulate -480 on the strict upper triangle of the
                        # diagonal 128x128 square: out[m,n] += -480*utri[n,m]
                        for hh in range(2):
                            d0 = off if hh == 0 else 512
                            nc.tensor.matmul(
                                spair[:, d0 : d0 + 128],
                                utri_sb,
                                negd_sb,
                                start=False,
                                stop=True,
                                skip_group_check=True,
                            )
                    s_tiles[J] = (spair, off)

                if PIPE:
                    pace = max(1, math.ceil(len(filler.q) / max(1, n_j)))
                else:
                    filler.drain_all()
                    pace = 0
                emit_S(0)
                for J in range(n_j):
                    filler.drain(pace)
                    if J + 1 < n_j:
                        emit_S(J + 1)
                    spair, off = s_tiles.pop(J)
                    w = CH - off
                    b_sl = [slice(off, CH), slice(512, 512 + w)]
                    e_pair = est_pool.tile([128, 1024], BF16, name="e_t", tag="e")
                    # with fp8 q/k the weights carry a x64 scale each, so S
                    # arrives x4096; fold the exact 2^-12 descale into exp
                    nc.scalar.activation(
                        e_pair[:, off : 512 + w],
                        spair[:, off : 512 + w],
                        Exp,
                        scale=0.125 / 4096.0 if FP8QK else 0.125,
                    )
                    r = J - c * JT
                    if (
                        not MASKMM
                        and mask_mode == "causal"
                        and 0 <= r < JT
                    ):
                        for hh in range(2):
                            d0 = b_sl[hh].start
                            nc.vector.tensor_mul(
                                e_pair[:, d0 : d0 + 128],
                                e_pair[:, d0 : d0 + 128],
                                tri_sb,
                            )
                    if mask_mode == "general":
                        for hh in range(2):
                            nc.vector.tensor_mul(
                                e_pair[:, b_sl[hh]],
                                e_pair[:, b_sl[hh]],
                                m_tiles[J][:, :CH],
                            )
                    for hh in range(2):
                        h = 2 * pr + hh
                        nc.tensor.matmul(
                            av_t[hh][:, off:CH],
                            v_sb[:, J, h * 65 : (h + 1) * 65],
                            e_pair[:, b_sl[hh]],
                            start=(J == 0),
                            stop=(J == n_j - 1),
                        )
                # epilogue: evacuate each head to a base-0 staging tile and
                # the two l rows into the half-chunk gather tile at
                # quadrant-aligned partitions (engine writes must start at
                # partition 0/32/64/96).
                zs = []
                # high priority: these copies free the av PSUM pair that the
                # NEXT unit's first AV matmul WAR-waits on; jump them ahead
                # of the division cluster in the DVE queue
                with tc.high_priority():
                    for hh in range(2):
                        z = zpool.tile([64, 512], BF16, name="z", tag="z")
                        nc.vector.tensor_copy(z[:, :CH], av_t[hh][0:64, :CH])
                        zs.append(z)
                        p0 = 64 * (pr % 2) + 32 * hh
                        nc.vector.tensor_copy(
                            Lc[p0 : p0 + 1, :CH], av_t[hh][64:65, :CH]
                        )
                if pending is not None:
                    # previous half-chunk's softmax-division cluster: emitted
                    # after this unit's drains and PSUM-freeing copies so its
                    # DVE backlog gates as little PE work as possible
                    pending()
                if mask_mode == "general":
                    del m_tiles
                return zs

            # ---- schedule ----------------------------------------------
            filler = _Filler()
            units = [(c, pr) for c in range(NCH) for pr in range(4)]

            # prologue: v tiles for chunk 0 (all tiles unless causal), q/k for
            # the first two units
            n_v_pro = JT if mask_mode == "causal" else NT
            for tt in range(n_v_pro):
                for th in v_piece(tt):
                    th()
            for c, pr in units[: min(2, len(units))]:
                for th in qk_piece("q", c, pr):
                    th()
                for th in qk_piece("k", c, pr):
                    th()

            qk_markers = {}
            v_markers = {}
            qk_stream = units[2:]
            v_next = n_v_pro  # next v tile to enqueue

            def make_division(c, z_pair, Lc):
                """Closure emitting the half-chunk softmax-division cluster
                (on DVE; the ACT Ln/Exp route flip-flops activation table
                sets with the attention Exp -- 1.3us reload each time).  The
                4 l rows sit at partitions 0/32/64/96; in-between lanes hold
                1.0 and are never read."""
                prs = list(z_pair.keys())
                # the very last division runs at drain time with ScalarE
                # still crunching the final unit's exp backlog -- spread the
                # Linv rows on the (then-idle) DVE instead so the tail
                # doesn't wait on the ACT queue
                spread_dve = c == NCH - 1 and prs[-1] == 3

                def emit():
                    Linv = lpool.tile([128, 512], F32, name="Linv", tag="linv")
                    # custom-DVE bit-trick reciprocal, ~5x faster than the
                    # iterative divide; HW-verified on this exact [0:97] AP
                    nc.vector.reciprocal_approx_fast(
                        Linv[0:97, :CH], Lc[0:97, :CH]
                    )
                    cs = slice(c * CH, (c + 1) * CH)
                    for pr2 in prs:
                        for hh in range(2):
                            p0 = 64 * (pr2 % 2) + 32 * hh
                            # the GPSIMD broadcast ucode ignores AP partition
                            # offsets on HW (reads p0, writes from p0), so
                            # spread each Linv row to a partition-0 tile
                            # first; row 0 can be read in place.  The spreads
                            # run on ScalarE (its copy shares the exp table
                            # set) to keep the DVE queue short here -- these
                            # only gate the deferred broadcasts.
                            li = lpool.tile([1, 512], BF16, name="li", tag="li")
                            if spread_dve:
                                nc.vector.tensor_copy(
                                    li[0:1, :CH], Linv[p0 : p0 + 1, :CH]
                                )
                            else:
                                nc.scalar.copy(
                                    li[0:1, :CH], Linv[p0 : p0 + 1, :CH]
                                )
                            li_ap = li[0:1, :CH]
                            hs = slice(hh * 64, (hh + 1) * 64)
                            if spread_dve:
                                # drain time: the PE is idle and ps_px free;
                                # broadcast via a K=1 ones-matmul (213ns)
                                # instead of 4 serial 1.1us GPSIMD broadcasts
                                lbc_ps = ps_px.tile(
                                    [128, 512], F32, name="lbc_ps", tag="px"
                                )
                                nc.tensor.matmul(
                                    lbc_ps[0:64, :CH],
                                    ones128[0:1, 0:64],
                                    li_ap,
                                    start=True,
                                    stop=True,
                                )
                                nc.vector.tensor_mul(
                                    concat_sb[hs, pr2, cs],
                                    z_pair[pr2][hh][:, :CH],
                                    lbc_ps[0:64, :CH],
                                )
                            else:
                                lbc = lbcp.tile(
                                    [64, 512], BF16, name="lbc", tag="lbc"
                                )
                                nc.gpsimd.partition_broadcast(
                                    lbc[:, :CH], li_ap, channels=64
                                )
                                nc.vector.tensor_mul(
                                    concat_sb[hs, pr2, cs],
                                    z_pair[pr2][hh][:, :CH],
                                    lbc[:, :CH],
                                )

                return emit

            pending_div = None
            for u, (c, pr) in enumerate(units):
                # enqueue filler due soon (outproj last so its matmuls drain
                # late in the J loop, after the previous half-chunk's
                # division cluster has emitted its concat multiplies)
                if u < len(qk_stream):
                    c2, pr2 = qk_stream[u]
                    m = filler.add(qk_piece("q", c2, pr2))
                    m = filler.add(qk_piece("k", c2, pr2))
                    qk_markers[(c2, pr2)] = m
                if v_next < NT and u >= 1:
                    # enqueue one unit later than strictly possible: the
                    # cc>=1 xT chunks land ~40-50us in, and an eagerly
                    # drained v matmul at the PE queue head would stall
                    # everything behind it on the DMA
                    m = filler.add(v_piece(v_next))
                    v_markers[v_next] = m
                    v_next += 1
                # outproj tile u-5: shifted one unit past the (c-1, pr-1)
                # ready point so its matmuls are always emitted after the
                # (deferred) division cluster that writes its concat inputs
                if 0 <= u - 5 < (NCH - 1) * JT:
                    filler.add(outproj_piece(u - 5))
                if u == len(units) - 1 and (NCH - 1) * JT - 1 >= 0:
                    # last unit also drains the final previous-chunk tile
                    filler.add(outproj_piece((NCH - 1) * JT - 1))

                # deadlines: q/k of this unit, v tiles of this chunk
                if (c, pr) in qk_markers:
                    filler.drain_until(qk_markers[(c, pr)])
                vt_needed = (c + 1) * JT - 1 if mask_mode == "causal" else NT - 1
                if vt_needed in v_markers:
                    filler.drain_until(v_markers[vt_needed])

                if pr % 2 == 0:
                    Lc = lpool.tile([128, 512], F32, name="Lc", tag="lc")
                    # initialize so the reciprocal over [0:97] never sees
                    # garbage; same queue as the l-copies so ordering is by
                    # emission
                    nc.vector.memset(Lc, 1.0)
                    z_pair = {}
                z_pair[pr] = emit_unit(c, pr, filler, Lc, pending=pending_div)
                pending_div = None

                if pr % 2 == 1:
                    pending_div = make_division(c, z_pair, Lc)

            # drain: final division, remaining filler, last outproj tiles
            if pending_div is not None:
                pending_div()
            filler.drain_all()
            for it in range((NCH - 1) * JT, NCH * JT):
                for th in outproj_piece(it):
                    th()
            if mask_mode == "general":
                _mt_cm.__exit__(None, None, None)
    nc.finalize()
    return nc


# ---------------------------------------------------------------------------
# Optional NTFF profiling (test.py sets TRACE=True). Registers the missing
# antenv.axon_hooks module so run_bass_kernel_spmd's trace path works.
TRACE = False
LAST_EXEC_TIME_NS = None
LAST_RESULTS = None


def _ensure_ntff_hook():
    import sys as _sys
    import types as _types

    if "antenv.axon_hooks" in _sys.modules:
        return
    mod = _types.ModuleType("antenv.axon_hooks")
    state = {"hook": None}
    mod.set_axon_ntff_profile_hook = lambda h: state.__setitem__("hook", h)
    mod.get_axon_ntff_profile_hook = lambda: state["hook"]
    _sys.modules["antenv.axon_hooks"] = mod
    import antenv

    antenv.axon_hooks = mod
    try:
        from trn_agent_boot.trn_boot import _ntff_profile_via_ctypes

        hook = _ntff_profile_via_ctypes("/opt/axon/libaxon_pjrt.so")
        if hook is not None:
            mod.set_axon_ntff_profile_hook(hook)
    except Exception:
        pass


_PROGRAM_CACHE = {}


def _get_program(T, mask_mode, has_bias):
    key = (T, mask_mode, has_bias)
    if key not in _PROGRAM_CACHE:
        _PROGRAM_CACHE[key] = build_core_program(T, mask_mode, has_bias)
    return _PROGRAM_CACHE[key]


def _mask_mode_of(mask):
    m = np.asarray(mask)
    if m.all():
        return "full"
    T = m.shape[0]
    tril = np.tril(np.ones((T, T), dtype=bool))
    if np.array_equal(m.astype(bool), tril):
        return "causal"
    return "general"


def kernel(x, mask, Wq, bq, Wk, bk, Wv, bv, Wo, bo):
    x = np.asarray(x)
    B, T, D_ = x.shape
    H = Wq.shape[0]
    assert D_ == D and H == 16
    mask_mode = _mask_mode_of(mask)
    has_bias = bool(
        np.any(np.asarray(bq)) or np.any(np.asarray(bk)) or np.any(np.asarray(bv))
    )
    nc = _get_program(T, mask_mode, has_bias)

    import os as _os

    fp8qk = _os.environ.get("K_FP8QK", "0") == "1" and not has_bias
    utri = np.triu(np.ones((128, 128), dtype=np.float32), 1).astype(nbf16)
    # -480: large enough that exp((S-480)/8) ~ 4e-26 ~ 0 in bf16, small
    # enough to stay inside the HW ACT exp spline's defined input range
    # (exp of ~-1e8 returns NaN on real hardware, unlike the simulator).
    # With fp8 q/k the S accumulator carries a x4096 scale; so must the mask.
    negd = (
        np.eye(128, dtype=np.float32) * (-480.0 * (4096.0 if fp8qk else 1.0))
    ).astype(nbf16)
    tri = np.triu(np.ones((128, 128), dtype=np.float32)).astype(nbf16)
    f8 = ml_dtypes.float8_e4m3fn
    if mask_mode == "general":
        maskT = np.ascontiguousarray(np.asarray(mask).T.astype(np.float32)).astype(
            nbf16
        )

    in_maps = []
    for core in range(8):
        b, g = core // 2, core % 2
        hsl = slice(g * HL, (g + 1) * HL)
        # (h, d, e) -> (d, h*e)
        wq = np.ascontiguousarray(
            np.transpose(np.asarray(Wq)[hsl], (1, 0, 2)).reshape(D, 512)
        ).astype(nbf16)
        wk = np.ascontiguousarray(
            np.transpose(np.asarray(Wk)[hsl], (1, 0, 2)).reshape(D, 512)
        ).astype(nbf16)
        wv = np.ascontiguousarray(
            np.transpose(np.asarray(Wv)[hsl], (1, 0, 2)).reshape(D, 512)
        ).astype(nbf16)
        wo = np.ascontiguousarray(np.asarray(Wo)[:, g * 512 : (g + 1) * 512].T).astype(
            nbf16
        )
        xTb = np.ascontiguousarray(x[b].T)
        im = {
            "xT": xTb.astype(nbf16),
            "wv": wv,
            "wo": wo,
            "utri": utri,
            "negd": negd,
            "tri": tri,
        }
        if fp8qk:
            im["x8"] = np.clip(xTb, -240, 240).astype(f8)
            im["wq8"] = np.clip(
                np.transpose(np.asarray(Wq)[hsl], (1, 0, 2)).reshape(D, 512) * 64.0,
                -240,
                240,
            ).astype(f8)
            im["wk8"] = np.clip(
                np.transpose(np.asarray(Wk)[hsl], (1, 0, 2)).reshape(D, 512) * 64.0,
                -240,
                240,
            ).astype(f8)
        else:
            im["wq"] = wq
            im["wk"] = wk
        if mask_mode == "general":
            im["maskT"] = maskT
        if has_bias:
            im["wqb"] = np.asarray(bq)[hsl].reshape(1, 512).astype(nbf16)
            im["wkb"] = np.asarray(bk)[hsl].reshape(1, 512).astype(nbf16)
            im["wvb"] = np.asarray(bv)[hsl].reshape(1, 512).astype(nbf16)
        in_maps.append(im)

    global LAST_EXEC_TIME_NS, LAST_RESULTS
    if TRACE:
        _ensure_ntff_hook()
    res = run_bass_kernel_spmd(nc, in_maps, core_ids=list(range(8)), trace=TRACE)
    LAST_RESULTS = res
    if TRACE:
        LAST_EXEC_TIME_NS = res.exec_time_ns
    out = np.empty((B, T, D), dtype=np.float32)
    bo_f = np.asarray(bo, dtype=np.float32)
    for b in range(B):
        out[b] = res.results[2 * b]["y"] + res.results[2 * b + 1]["y"] + bo_f
    return out
